# revision 1
# baseline (speedup 1.0000x reference)
"""DCN-FPN Trainium2 kernel (nn_DCNFPN).

Sharding: 8 cores = 4 images x 2 row-halves. Each core computes rows
[g0, g0+23] of every 40-row intermediate (g0 = 0 top / 16 bottom), with
shrinking-validity redundancy so no cross-core communication is needed:
the correct-row front shrinks by 1 row per DCN iteration and we carry 4
spare rows; host keeps rows 0..19 (top) / 20..39 (bottom) of the output.

Per DCN call (4 calls: levels 0,1,0,1):
  - offset conv (3x3, 256->48) as 36 bf16 matmuls accumulating in PSUM
  - small math on [64,480] tiles (p = yx*32 + rcb*16 + tap) computes
    bilinear corner row-pair indices + 4 slot weights (mask/validity
    folded in; x-OOB handled by slot-remap E0/E1/F0 logic)
  - per tap (16 chunks): dma_gather of top/bot 2-pixel row pairs
    (bf16, elem 1KB) from the pixel-major feature table in DRAM,
    weight broadcast DMA, 7 DVE ops to combine corners, 8 matmuls
    accumulating dc in PSUM
  - f += relu(dc) in fp32 master, bf16 shadow for matmuls
Final: residual conv + fh, store [256, 960] fp32.

Sample enumeration: i = tap*960 + rc, rc = rcb*480 + c (rcb in {0,1}).
Gather idx layout [i%16, i//16] == [rc%16, tap*60 + rcb*30 + c//16].
"""
import sys
sys.path.insert(0, "/opt/trn_rl_repo")

from contextlib import ExitStack
import numpy as np
import ml_dtypes

import os
import bass_rust
import concourse.bass as bass
import concourse.bacc as bacc
import concourse.mybir as mybir
import concourse.tile as tile

F32 = mybir.dt.float32
BF16 = mybir.dt.bfloat16
I16 = mybir.dt.int16
I32 = mybir.dt.int32
A = mybir.AluOpType
AF = mybir.ActivationFunctionType

B, C, HOUT = 4, 256, 40
CONFIGS = [(4, 2, 1, 1), (4, 4, 3, 3)]   # (k, stride, pad, dil)
HIN = [80, 160]                          # per level l=0 (f1), l=1 (f0)
ROWS = 24                                # out rows per core per call
RC = ROWS * HOUT                         # 960
NT = 16                                  # taps
CALLS = [0, 1, 0, 1]
FW = 42                                  # padded f width
FR = 26                                  # f window rows
FSZ = FR * FW                            # 1092


def ap_with(ap, dims, offset_elems=None):
    v = ap.copy()
    if offset_elems is not None:
        v = v  # offset handled by caller slicing
    v.ap = bass_rust.VecI64Pair(dims)
    return v


def build_program():
    nc = bacc.Bacc("TRN2", target_bir_lowering=False, debug=False)

    dt = {}

    def din(name, shape, dtype=F32):
        dt[name] = nc.dram_tensor(name, shape, dtype, kind="ExternalInput").ap()

    din("fp0", [HIN[1] * HIN[1] + 1, C], BF16)
    din("fp1", [HIN[0] * HIN[0] + 1, C], BF16)
    din("finit", [C, FSZ], F32)
    din("fh", [128, 2 * RC], F32)
    din("byx", [64, 2 * 480], F32)
    din("hi0", [64, 2], F32)
    din("com_w", [128, 2 * 9 * 2 * 48], BF16)
    din("com_b", [48, 2], F32)
    din("dcn_w", [2, 128, NT * 2 * 2 * 128], BF16)
    din("dcn_b", [128, 4], F32)
    din("res_w", [128, 9 * 2 * 2 * 128], BF16)
    din("res_b", [128, 2], F32)
    out_d = nc.dram_tensor("out", [C, RC], F32, kind="ExternalOutput").ap()

    with tile.TileContext(nc) as tc, ExitStack() as ctx:
        build_body(nc, tc, ctx, dt, out_d)
    nc.compile()
    return nc


def build_body(nc, tc, ctx, dt, out_d):
    cst = ctx.enter_context(tc.tile_pool(name="cst", bufs=1))
    s64p = ctx.enter_context(tc.tile_pool(name="s64p", bufs=10))
    s32p = ctx.enter_context(tc.tile_pool(name="s32p", bufs=6))
    i32p = ctx.enter_context(tc.tile_pool(name="i32p", bufs=2))
    smi = ctx.enter_context(tc.tile_pool(name="smi", bufs=2))
    omp = ctx.enter_context(tc.tile_pool(name="omp", bufs=1))
    wgt = ctx.enter_context(tc.tile_pool(name="wgt", bufs=1))
    gat = ctx.enter_context(tc.tile_pool(name="gat", bufs=2))
    wbp = ctx.enter_context(tc.tile_pool(name="wbp", bufs=2))
    pp = ctx.enter_context(tc.tile_pool(name="pp", bufs=4))
    qp = ctx.enter_context(tc.tile_pool(name="qp", bufs=3))
    sp = ctx.enter_context(tc.tile_pool(name="sp", bufs=4))
    fup = ctx.enter_context(tc.tile_pool(name="fup", bufs=1))
    pso = ctx.enter_context(tc.tile_pool(name="pso", bufs=1, space="PSUM"))
    psd = ctx.enter_context(tc.tile_pool(name="psd", bufs=1, space="PSUM"))
    drp = ctx.enter_context(tc.tile_pool(name="drp", bufs=2, space="DRAM"))

    # ---- persistent loads -------------------------------------------------
    com_t = cst.tile([128, 2 * 9 * 2 * 48], BF16, tag="com")
    nc.sync.dma_start(com_t[:], dt["com_w"])
    com_v = com_t[:].rearrange("p (l t i o) -> p l t i o", l=2, t=9, i=2, o=48)

    byx_t = cst.tile([64, 2 * 480], F32, tag="byx")
    nc.sync.dma_start(byx_t[:], dt["byx"])
    hi0_t = cst.tile([64, 2], F32, tag="hi0")
    nc.sync.dma_start(hi0_t[:], dt["hi0"])
    comb_t = cst.tile([48, 2], F32, tag="comb")
    nc.sync.dma_start(comb_t[:], dt["com_b"])
    dcnb_t = cst.tile([128, 4], F32, tag="dcnb")
    nc.sync.dma_start(dcnb_t[:], dt["dcn_b"])
    resb_t = cst.tile([128, 2], F32, tag="resb")
    nc.sync.dma_start(resb_t[:], dt["res_b"])
    fh_t = cst.tile([128, 2 * RC], F32, tag="fh")
    nc.sync.dma_start(fh_t[:], dt["fh"])

    fmas, fsh = [], []
    for h in range(2):
        fm = cst.tile([128, FSZ], F32, tag=f"fmas{h}")
        nc.sync.dma_start(fm[:], dt["finit"][128 * h:128 * (h + 1), :])
        fs = cst.tile([128, FSZ], BF16, tag=f"fsh{h}")
        nc.vector.tensor_copy(fs[:], fm[:])
        fmas.append(fm)
        fsh.append(fs)

    fp_ap = {0: dt["fp1"], 1: dt["fp0"]}

    # ---- per-call ---------------------------------------------------------
    for ci, lvl in enumerate(CALLS):
        k_, st_, pad_, dil_ = CONFIGS[lvl]
        Hin = Win = HIN[lvl]

        # per-call DCN weights
        dcn_t = wgt.tile([128, NT * 2 * 2 * 128], BF16, tag="dcn")
        nc.sync.dma_start(dcn_t[:], dt["dcn_w"][lvl])
        dcn_v = dcn_t[:].rearrange("p (k i o q) -> p k i o q", k=NT, i=2, o=2, q=128)

        # offset conv: om_ps [48, 960]
        om_ps = pso.tile([48, 1024], F32, tag="omps")
        conv3x3(nc, fsh, lambda ti, ih: com_v[:, lvl, ti, ih], om_ps)

        om01 = omp.tile([32, RC], F32, tag="om01")
        omv0 = om_ps[0:32, :].rearrange("p (z c) -> p z c", z=2)[:, :, 0:480]
        nc.scalar.activation(om01[:], omv0, AF.Identity,
                             bias=comb_t[0:32, lvl:lvl + 1])
        m16 = omp.tile([16, RC], F32, tag="m16")
        omv1 = om_ps[32:48, :].rearrange("p (z c) -> p z c", z=2)[:, :, 0:480]
        nc.scalar.activation(m16[:], omv1, AF.Sigmoid,
                             bias=comb_t[32:48, lvl:lvl + 1])

        # shuffle into [64,480] (p = yx*32 + rcb*16 + t) / [32,480]
        pos0 = s64p.tile([64, 480], F32, tag="s64")
        for yx in range(2):
            for rcb in range(2):
                nc.sync.dma_start(
                    pos0[yx * 32 + rcb * 16: yx * 32 + rcb * 16 + 16, :],
                    om01[yx * 16:yx * 16 + 16, rcb * 480:(rcb + 1) * 480])
        m32 = s32p.tile([32, 480], F32, tag="s32")
        for rcb in range(2):
            nc.sync.dma_start(m32[rcb * 16:rcb * 16 + 16, :],
                              m16[:, rcb * 480:(rcb + 1) * 480])

        # ---- small math ----
        cnt = [0]

        def t64():
            cnt[0] += 1
            return s64p.tile([64, 480], F32, tag="s64", name=f"t64_{ci}_{cnt[0]}")

        def t32():
            cnt[0] += 1
            return s32p.tile([32, 480], F32, tag="s32", name=f"t32_{ci}_{cnt[0]}")

        pos = t64()
        nc.vector.tensor_tensor(pos[:], pos0[:],
                                byx_t[:, lvl * 480:(lvl + 1) * 480], A.add)
        sh = t64()
        nc.vector.tensor_scalar(sh[:], pos[:], 1024.0, None, A.add)
        i32t = i32p.tile([64, 480], I32, tag="i32")
        nc.vector.tensor_copy(i32t[:], sh[:])
        ff = t64()
        nc.vector.tensor_copy(ff[:], i32t[:])
        gt = t64()
        nc.vector.tensor_tensor(gt[:], ff[:], sh[:], A.is_gt)
        fl = t64()
        nc.vector.scalar_tensor_tensor(fl[:], ff[:], -1024.0, gt[:], A.add, A.subtract)
        frac = t64()
        nc.vector.tensor_tensor(frac[:], pos[:], fl[:], A.subtract)
        c0 = t64()
        nc.vector.tensor_scalar(c0[:], fl[:], 0.0, hi0_t[:, lvl:lvl + 1],
                                A.max, A.min)
        t1 = t64()
        nc.vector.tensor_scalar(t1[:], fl[:], 1.0, None, A.add)
        cc1 = t64()
        nc.vector.tensor_scalar(cc1[0:32, :], t1[0:32, :], 0.0, float(Hin - 1),
                                A.max, A.min)
        nc.vector.tensor_copy(cc1[32:64, :], c0[32:64, :])
        V0 = t64()
        nc.vector.tensor_tensor(V0[:], c0[:], fl[:], A.is_equal)
        V1 = t64()
        nc.vector.tensor_tensor(V1[:], cc1[:], t1[:], A.is_equal)
        # x-side ops stay on base-partition 32 (walrus: TT inputs must share
        # base partition); results copied down to base 0 where y-side needs them
        psp = t64()
        nc.vector.tensor_scalar(psp[32:64, :], c0[32:64, :], 1.0, None, A.add)
        F0 = t64()
        nc.vector.tensor_tensor(F0[32:64, :], psp[32:64, :], fl[32:64, :], A.is_equal)
        u = t64()
        nc.vector.tensor_scalar(u[:], frac[:], -1.0, 1.0, A.mult, A.add)
        P0 = t64()
        nc.vector.tensor_tensor(P0[:], u[:], V0[:], A.mult)
        P1 = t64()
        nc.vector.tensor_tensor(P1[:], frac[:], V1[:], A.mult)
        xsA = t64()
        nc.vector.tensor_tensor(xsA[32:64, :], P0[32:64, :], P1[32:64, :], A.add)
        xt = t64()
        nc.vector.tensor_tensor(xt[32:64, :], F0[32:64, :], u[32:64, :], A.mult)
        xt2 = t64()
        nc.vector.tensor_tensor(xt2[32:64, :], V0[32:64, :], frac[32:64, :], A.mult)
        xsB = t64()
        nc.vector.tensor_tensor(xsB[32:64, :], xt[32:64, :], xt2[32:64, :], A.add)
        xs0 = t32()
        nc.vector.tensor_copy(xs0[:], xsA[32:64, :])
        xs1 = t32()
        nc.vector.tensor_copy(xs1[:], xsB[32:64, :])
        A0 = t32()
        nc.vector.tensor_tensor(A0[:], P0[0:32, :], m32[:], A.mult)
        A1 = t32()
        nc.vector.tensor_tensor(A1[:], P1[0:32, :], m32[:], A.mult)

        # wall [32, (slot j, c512)] bf16, j = A(top,pix0) B(top,pix1) C D (bot)
        # 512-col blocks: 480 real + 32 zero pad (sample space padded to 1024/tap)
        wall = smi.tile([32, 4 * 512], BF16, tag="wall")
        nc.vector.memset(wall[:], 0.0)
        nc.vector.tensor_tensor(wall[:, 0 * 512:0 * 512 + 480], A0[:], xs0[:], A.mult)
        nc.vector.tensor_tensor(wall[:, 1 * 512:1 * 512 + 480], A0[:], xs1[:], A.mult)
        nc.vector.tensor_tensor(wall[:, 2 * 512:2 * 512 + 480], A1[:], xs0[:], A.mult)
        nc.vector.tensor_tensor(wall[:, 3 * 512:3 * 512 + 480], A1[:], xs1[:], A.mult)
        wdram = drp.tile([32, 4 * 512], BF16, tag="wdram")
        nc.sync.dma_start(wdram[:], wall[:])
        wflat = wdram[:].rearrange("p f -> (p f)")

        # gather indices (f32 -> i16), wrap via DRAM, replicate to 128p
        psx = t32()
        nc.vector.tensor_copy(psx[:], c0[32:64, :])
        gyt = t32()
        nc.vector.scalar_tensor_tensor(gyt[:], c0[0:32, :], float(Win),
                                       psx[:], A.mult, A.add)
        gyb = t32()
        nc.vector.scalar_tensor_tensor(gyb[:], cc1[0:32, :], float(Win),
                                       psx[:], A.mult, A.add)
        idxs = []
        for gsrc in (gyt, gyb):
            i16t = smi.tile([32, 480], I16, tag="i16")
            nc.vector.tensor_copy(i16t[:], gsrc[:])
            dflat = drp.tile([32, 480], I16, tag="dfl")
            nc.sync.dma_start(dflat[:], i16t[:])
            # wrap_z[p', t*30+cc] = dflat[(z*16+t)*480 + cc*16 + p']
            dfv = dflat[:].rearrange("p c -> (p c)")
            rep = smi.tile([128, NT * 64], I16, tag="rep")
            nc.vector.memset(rep[:], 0)
            repv = rep[:].rearrange("p (t z cc) -> p t z cc", t=NT, z=2, cc=32)
            for z in range(2):
                wrap = smi.tile([16, NT * 30], I16, tag="wrap",
                                name=f"wrap_{ci}_{len(idxs)}_{z}")
                src = dfv[z * 16 * 480:(z + 1) * 16 * 480]
                src = src.rearrange("(tc p) -> p tc", p=16)
                nc.scalar.dma_start(wrap[:], src)
                wv_ = wrap[:].rearrange("p (t cc) -> p t cc", t=NT)
                nc.scalar.dma_start(repv[0:16, :, z, 0:30], wv_)
            # log-double rows 0:16 -> 128 (full contiguous rows incl pad)
            for k in (16, 32, 64):
                nc.scalar.dma_start(rep[k:2 * k, :], rep[0:k, :])
            idxs.append(rep)

        # dc accumulator [2][128, 960]
        dcs = [psd.tile([128, 1024], F32, tag=f"dc{oh}", name=f"dc_{ci}_{oh}")
               for oh in range(2)]

        fpv = fp_ap[lvl].copy()
        fpv.ap = bass_rust.VecI64Pair([[C, Hin * Win], [1, 2 * C]])

        RCP = 1024
        for t in range(NT):
            # weight broadcast: wb free (j, z, c512) <- wdram (z*16+t)*2048+j*512+c
            wb = wbp.tile([128, 4 * RCP], BF16, tag="wb")
            wb4 = wb[:].rearrange("p (j z c) -> p j z c", j=4, z=2, c=512)
            for z in range(2):
                src = wflat[(z * 16 + t) * 2048:(z * 16 + t) * 2048 + 1].copy()
                src.ap = bass_rust.VecI64Pair([[0, 128], [512, 4], [1, 512]])
                nc.sync.dma_start(wb4[:, :, z, :], src)
            wbv = wb[:].rearrange("p (j zc) -> p j zc", j=4)

            gts = []
            for corner in range(2):
                g = gat.tile([128, 4 * RCP], BF16, tag=f"g{corner}")
                gv = g[:].rearrange("p (j i) -> p j i", j=4)
                nc.gpsimd.dma_gather(gv, fpv, idxs[corner][:, t * 64:(t + 1) * 64],
                                     RCP, RCP, 2 * C, elem_step=C,
                                     transpose=True, single_packet=False)
                gts.append(g[:].rearrange("p (pix hl i) -> p hl pix i",
                                          pix=2, hl=2))

            ps_ = []
            for corner in range(2):
                for hilo in range(2):
                    p = pp.tile([128, 2 * RCP], BF16, tag="p")
                    pv = p[:].rearrange("p (j i) -> p j i", j=2)
                    gsl = gts[corner][:, hilo]                 # [128, 2, 1024]
                    wsl = wbv[:, 2 * corner:2 * corner + 2, :]  # [128, 2, 1024]
                    nc.vector.tensor_tensor(pv, gsl, wsl, A.mult)
                    ps_.append(p)
            ss = []
            for hilo in range(2):
                q = qp.tile([128, 2 * RCP], BF16, tag="q")
                nc.vector.tensor_tensor(q[:], ps_[hilo][:], ps_[2 + hilo][:], A.add)
                s_ = sp.tile([128, RCP], BF16, tag="s")
                nc.vector.tensor_tensor(s_[:], q[:, 0:RCP], q[:, RCP:2 * RCP], A.add)
                ss.append(s_)

            for oh in range(2):
                for ih in range(2):
                    for nh in range(2):
                        nc.tensor.matmul(
                            dcs[oh][:, nh * 512:(nh + 1) * 512],
                            dcn_v[:, t, ih, oh],
                            ss[ih][:, nh * 512:(nh + 1) * 512],
                            start=(t == 0 and ih == 0),
                            stop=(t == NT - 1 and ih == 1))

        # f update: f += relu(dc + b)
        for h in range(2):
            rel = fup.tile([128, RC], F32, tag="rel")
            dcv = dcs[h][:].rearrange("p (z c) -> p z c", z=2)[:, :, 0:480]
            nc.scalar.activation(rel[:], dcv, AF.Relu,
                                 bias=dcnb_t[:, 2 * lvl + h:2 * lvl + h + 1])
            fiv = fmas[h][:].rearrange("p (r c) -> p r c", c=FW)[:, 1:25, 1:41]
            rv = rel[:].rearrange("p (r c) -> p r c", c=HOUT)
            nc.vector.tensor_tensor(fiv, fiv, rv, A.add)
            fsv = fsh[h][:].rearrange("p (r c) -> p r c", c=FW)[:, 1:25, 1:41]
            nc.vector.tensor_copy(fsv, fiv)

    # ---- residual conv + fh ----------------------------------------------
    res_t = wgt.tile([128, 9 * 2 * 2 * 128], BF16, tag="dcn")
    nc.sync.dma_start(res_t[:], dt["res_w"])
    res_v = res_t[:].rearrange("p (t i o q) -> p t i o q", t=9, i=2, o=2)
    for oh in range(2):
        rps = psd.tile([128, 1024], F32, tag=f"dc{oh}")
        conv3x3(nc, fsh, lambda ti, ih, oh=oh: res_v[:, ti, ih, oh], rps)
        ot = fup.tile([128, RC], F32, tag="ot")
        rpv = rps[:].rearrange("p (z c) -> p z c", z=2)[:, :, 0:480]
        nc.scalar.activation(ot[:], rpv, AF.Identity, bias=resb_t[:, oh:oh + 1])
        nc.vector.tensor_tensor(ot[:], ot[:], fh_t[:, oh * RC:(oh + 1) * RC], A.add)
        nc.sync.dma_start(out_d[128 * oh:128 * (oh + 1), :], ot[:])


def conv3x3(nc, fsh, w_fn, out_ps):
    """3x3 stride-1 conv over the padded f window; out [cout, 960]."""
    taps = [(a, b) for a in (-1, 0, 1) for b in (-1, 0, 1)]
    for ti, (dy, dx) in enumerate(taps):
        for ih in range(2):
            rhs = fsh[ih][:].rearrange("p (r c) -> p r c", c=FW)
            for nh in range(2):
                nc.tensor.matmul(
                    out_ps[:, nh * 512:nh * 512 + 480],
                    w_fn(ti, ih),
                    rhs[:, 1 + dy + nh * 12:1 + dy + nh * 12 + 12,
                        1 + dx:1 + dx + 40],
                    start=(ti == 0 and ih == 0), stop=(ti == 8 and ih == 1))


# ===========================================================================
# host side
# ===========================================================================

def prep_core_inputs(inputs, b, half):
    """Per-core input map for image b, row-half `half` (0=top)."""
    g0 = 0 if half == 0 else 16
    f0 = np.asarray(inputs["f0"][b], np.float32)
    f1 = np.asarray(inputs["f1"][b], np.float32)
    f2 = np.asarray(inputs["f2"][b], np.float32)

    def pix_table(f):
        hw = f.shape[1] * f.shape[2]
        t = np.zeros((hw + 1, C), np.float32)
        t[:hw] = f.transpose(1, 2, 0).reshape(hw, C)
        return t.astype(ml_dtypes.bfloat16)

    finit = np.zeros((C, FR, FW), np.float32)
    for r in range(FR):
        gr = g0 - 1 + r
        if 0 <= gr < HOUT:
            finit[:, r, 1:41] = f2[:, gr, :]

    # fh as [128, (oh, rc)]
    fh0 = f2[:, g0:g0 + ROWS, :].reshape(C, RC)
    fh = np.concatenate([fh0[:128], fh0[128:]], axis=1)

    byx = np.zeros((2, 64, 480), np.float32)
    hi0 = np.zeros((2, 64, 1), np.float32)
    for lvl in range(2):
        k_, st_, pad_, dil_ = CONFIGS[lvl]
        Hin = HIN[lvl]
        rc = np.arange(480)
        for rcb in range(2):
            rr = (rcb * 480 + rc) // HOUT
            cc = (rcb * 480 + rc) % HOUT
            for t in range(NT):
                byx[lvl, rcb * 16 + t] = st_ * (g0 + rr) - pad_ + (t // k_) * dil_
                byx[lvl, 32 + rcb * 16 + t] = st_ * cc - pad_ + (t % k_) * dil_
        hi0[lvl, 0:32] = Hin - 1
        hi0[lvl, 32:64] = Hin - 2
    byx = byx.transpose(1, 0, 2).reshape(64, 2 * 480)
    hi0 = hi0.transpose(1, 0, 2).reshape(64, 2)

    perm = list(range(0, 32, 2)) + list(range(1, 32, 2)) + list(range(32, 48))
    com_w = np.zeros((2, 9, 2, 128, 48), np.float32)
    com_b = np.zeros((2, 48, 1), np.float32)
    dcn_w = np.zeros((2, NT, 2, 2, 128, 128), np.float32)
    dcn_b = np.zeros((2, 2, 128, 1), np.float32)
    for lvl in range(2):
        cw = np.asarray(inputs[f"com_w{lvl}"], np.float32)[perm]
        cb = np.asarray(inputs[f"com_b{lvl}"], np.float32)[perm]
        for ty in range(3):
            for tx in range(3):
                for ih in range(2):
                    com_w[lvl, ty * 3 + tx, ih] = \
                        cw[:, ih * 128:(ih + 1) * 128, ty, tx].T
        com_b[lvl, :, 0] = cb
        dw = np.asarray(inputs[f"dcn_w{lvl}"], np.float32)
        for k in range(NT):
            for ih in range(2):
                for oh in range(2):
                    dcn_w[lvl, k, ih, oh] = dw[oh * 128:(oh + 1) * 128,
                                               ih * 128:(ih + 1) * 128,
                                               k // 4, k % 4].T
        db = np.asarray(inputs[f"dcn_b{lvl}"], np.float32)
        dcn_b[lvl, 0, :, 0] = db[:128]
        dcn_b[lvl, 1, :, 0] = db[128:]
    rw = np.asarray(inputs["res_w"], np.float32)
    res_w = np.zeros((9, 2, 2, 128, 128), np.float32)
    for ty in range(3):
        for tx in range(3):
            for ih in range(2):
                for oh in range(2):
                    res_w[ty * 3 + tx, ih, oh] = rw[oh * 128:(oh + 1) * 128,
                                                    ih * 128:(ih + 1) * 128,
                                                    ty, tx].T
    rb = np.asarray(inputs["res_b"], np.float32)
    res_b = np.stack([rb[:128], rb[128:]], axis=1)  # [128, 2]

    # transpose weight stacks to [partition, ...] DRAM layouts
    com_w = com_w.transpose(3, 0, 1, 2, 4).reshape(128, -1)
    com_b = com_b.transpose(1, 0, 2).reshape(48, 2)
    dcn_w = dcn_w.transpose(0, 4, 1, 2, 3, 5).reshape(2, 128, -1)
    dcn_b = dcn_b.transpose(2, 0, 1, 3).reshape(128, 4)
    res_w = res_w.transpose(3, 0, 1, 2, 4).reshape(128, -1)

    return {
        "fp0": pix_table(f0),
        "fp1": pix_table(f1),
        "finit": finit.reshape(C, FSZ),
        "fh": fh.astype(np.float32),
        "byx": byx,
        "hi0": hi0,
        "com_w": com_w.astype(ml_dtypes.bfloat16),
        "com_b": np.ascontiguousarray(com_b),
        "dcn_w": np.ascontiguousarray(dcn_w).astype(ml_dtypes.bfloat16),
        "dcn_b": np.ascontiguousarray(dcn_b),
        "res_w": np.ascontiguousarray(res_w).astype(ml_dtypes.bfloat16),
        "res_b": np.ascontiguousarray(res_b).astype(np.float32),
    }


def assemble_output(results):
    out = np.zeros((B, C, HOUT, HOUT), np.float32)
    for b in range(B):
        top = np.asarray(results[2 * b]["out"]).reshape(C, ROWS, HOUT)
        bot = np.asarray(results[2 * b + 1]["out"]).reshape(C, ROWS, HOUT)
        out[b, :, 0:20, :] = top[:, 0:20, :]
        out[b, :, 20:40, :] = bot[:, 4:24, :]
    return out


_NC_CACHE = []


def kernel(**inputs):
    if not _NC_CACHE:
        _NC_CACHE.append(build_program())
    nc = _NC_CACHE[0]
    in_maps = [prep_core_inputs(inputs, b, half)
               for b in range(B) for half in range(2)]
    from concourse.bass_utils import run_bass_kernel_spmd
    r = run_bass_kernel_spmd(nc, in_maps, list(range(8)))
    return assemble_output(r.results)



# revision 18
# speedup vs baseline: 1.0760x; 1.0760x over previous
"""DCN-FPN Trainium2 kernel (nn_DCNFPN), v2.

Sharding: 8 cores = 4 images x 2 row-halves. Each core computes rows
[g0, g0+23] of every 40-row intermediate (g0 = 0 top / 16 bottom), with
shrinking-validity redundancy so no cross-core communication is needed;
host keeps rows 0..19 (top) / 20..39 (bottom) of the output.

Key structure (vs v1): the DRAM feature table packs the full 2x2
bilinear patch per entry -- entry (yy, xx) of an (H+1)x(W+1) grid holds
[f[yy-1,xx-1], f[yy-1,xx], f[yy,xx-1], f[yy,xx]] over 256 channels
(zero-filled out of bounds), 2 KB each.  One dma_gather per tap fetches
all four corners; OOB x/y handling collapses into table zeros plus one
per-axis clamp-indicator folded into the mask.  The four slot weights
(A0,A1)x(xs0,xs1) are broadcast to 128 partitions through the PE (ones
[1,128] matmul) and copied PSUM->SBUF bf16 by the Activation engine --
no DRAM round trip.  Corner combine: 2 in-place TT muls + q/s adds on
DVE (bf16, 2x mode), then 8 PSUM-accumulating matmuls per tap.

Per call: offset conv (36 mm) -> om activations -> shuffle -> small
math ([64,480]: floor/frac/clamp/valid; walls+idx on [32,480]) -> idx
i16 wrap via DRAM -> 16-tap pipeline -> f += relu(dc).
Final: residual conv + fh, store [256, 960] fp32.

Sample enumeration per tap: gather column i = 512*z + 16*cc + p
(z = rc//480, p = rc%16, cc = (rc%480)//16); columns 480:512 of each
512-block are pad (idx 0, ignored).
"""
import sys
sys.path.insert(0, "/opt/trn_rl_repo")

from contextlib import ExitStack
import numpy as np
import ml_dtypes

import bass_rust
import concourse.bass as bass
import concourse.bacc as bacc
import concourse.mybir as mybir
import concourse.tile as tile

F32 = mybir.dt.float32
BF16 = mybir.dt.bfloat16
I16 = mybir.dt.int16
I32 = mybir.dt.int32
A = mybir.AluOpType
AF = mybir.ActivationFunctionType

B, C, HOUT = 4, 256, 40
CONFIGS = [(4, 2, 1, 1), (4, 4, 3, 3)]   # (k, stride, pad, dil)
HIN = [80, 160]                          # per level l=0 (f1), l=1 (f0)
TW = [HIN[0] + 1, HIN[1] + 1]            # packed-table grid width per level
ROWS = 24                                # out rows per core per call
RC = ROWS * HOUT                         # 960
NT = 16                                  # taps
CALLS = [0, 1, 0, 1]
FW = 42                                  # padded f width
FR = 26                                  # f window rows
FSZ = FR * FW                            # 1092


def vp(ap, dims, doff=0):
    v = ap.copy()
    v.ap = bass_rust.VecI64Pair(dims)
    if doff:
        v.offset = v.offset + doff
    return v


def build_program():
    nc = bacc.Bacc("TRN2", target_bir_lowering=False, debug=False)

    dt = {}

    def din(name, shape, dtype=F32):
        dt[name] = nc.dram_tensor(name, shape, dtype, kind="ExternalInput").ap()

    din("fp0", [TW[1] * TW[1], 1024], BF16)   # level 1 packed table (f0)
    din("fp1", [TW[0] * TW[0], 1024], BF16)   # level 0 packed table (f1)
    din("finit", [C, FSZ], F32)
    din("fh", [128, 2 * RC], F32)
    din("byx", [64, 2 * 480], F32)
    din("hi0", [64, 2], F32)
    din("com_w", [128, 2 * 9 * 2 * 48], BF16)
    din("com_b", [48, 2], F32)
    din("dcn_w", [2, 128, NT * 2 * 2 * 128], BF16)
    din("dcn_b", [128, 4], F32)
    din("res_w", [128, 9 * 2 * 2 * 128], BF16)
    din("res_b", [128, 2], F32)
    out_d = nc.dram_tensor("out", [C, RC], F32, kind="ExternalOutput").ap()

    with tile.TileContext(nc) as tc, ExitStack() as ctx:
        build_body(nc, tc, ctx, dt, out_d)
    nc.compile()
    return nc


def build_body(nc, tc, ctx, dt, out_d):
    cst = ctx.enter_context(tc.tile_pool(name="cst", bufs=1))
    s64p = ctx.enter_context(tc.tile_pool(name="s64p", bufs=6))
    s32p = ctx.enter_context(tc.tile_pool(name="s32p", bufs=5))
    i32p = ctx.enter_context(tc.tile_pool(name="i32p", bufs=1))
    smi = ctx.enter_context(tc.tile_pool(name="smi", bufs=1))
    omp = ctx.enter_context(tc.tile_pool(name="omp", bufs=1))
    wgt = ctx.enter_context(tc.tile_pool(name="wgt", bufs=1))
    walp = ctx.enter_context(tc.tile_pool(name="walp", bufs=2))
    wrp = ctx.enter_context(tc.tile_pool(name="wrp", bufs=2))
    wbp = ctx.enter_context(tc.tile_pool(name="wbp", bufs=2))
    gat = ctx.enter_context(tc.tile_pool(name="gat", bufs=2))
    qp = ctx.enter_context(tc.tile_pool(name="qp", bufs=2))
    sp = ctx.enter_context(tc.tile_pool(name="sp", bufs=2))
    fup = ctx.enter_context(tc.tile_pool(name="fup", bufs=1))
    pso = ctx.enter_context(tc.tile_pool(name="pso", bufs=2, space="PSUM"))
    psd = ctx.enter_context(tc.tile_pool(name="psd", bufs=1, space="PSUM"))
    drp = ctx.enter_context(tc.tile_pool(name="drp", bufs=2, space="DRAM"))

    # ---- persistent loads -------------------------------------------------
    com_t = cst.tile([128, 2 * 9 * 2 * 48], BF16, tag="com")
    nc.sync.dma_start(com_t[:], dt["com_w"])
    com_v = com_t[:].rearrange("p (l t i o) -> p l t i o", l=2, t=9, i=2, o=48)

    byx_t = cst.tile([64, 2 * 480], F32, tag="byx")
    nc.sync.dma_start(byx_t[:], dt["byx"])
    hi0_t = cst.tile([64, 2], F32, tag="hi0")
    nc.sync.dma_start(hi0_t[:], dt["hi0"])
    comb_t = cst.tile([48, 2], F32, tag="comb")
    nc.sync.dma_start(comb_t[:], dt["com_b"])
    dcnb_t = cst.tile([128, 4], F32, tag="dcnb")
    nc.sync.dma_start(dcnb_t[:], dt["dcn_b"])
    resb_t = cst.tile([128, 2], F32, tag="resb")
    nc.sync.dma_start(resb_t[:], dt["res_b"])
    fh_t = cst.tile([128, 2 * RC], F32, tag="fh")
    nc.sync.dma_start(fh_t[:], dt["fh"])
    ones_t = cst.tile([1, 128], BF16, tag="ones")
    nc.vector.memset(ones_t[:], 1.0)

    # per-level DCN weights, loaded once
    dcn_ts = []
    for lvl in range(2):
        t_ = cst.tile([128, NT * 2 * 2 * 128], BF16, tag=f"dcn{lvl}")
        nc.sync.dma_start(t_[:], dt["dcn_w"][lvl])
        dcn_ts.append(t_[:].rearrange("p (k i o q) -> p k i o q",
                                      k=NT, i=2, o=2, q=128))

    fmas, fsh = [], []
    for h in range(2):
        fm = cst.tile([128, FSZ], F32, tag=f"fmas{h}")
        nc.sync.dma_start(fm[:], dt["finit"][128 * h:128 * (h + 1), :])
        fs = cst.tile([128, FSZ], BF16, tag=f"fsh{h}")
        nc.vector.tensor_copy(fs[:], fm[:])
        fmas.append(fm)
        fsh.append(fs)

    fp_ap = {0: dt["fp1"], 1: dt["fp0"]}

    # ---- per-call ---------------------------------------------------------
    for ci, lvl in enumerate(CALLS):
        Win = HIN[lvl]
        Wt = TW[lvl]
        dcn_v = dcn_ts[lvl]

        # offset conv: om_ps rows 0:48, (z,512)-chunked, 480 used
        om_ps = pso.tile([128, 1024], F32, tag="ps", name=f"omps_{ci}")
        conv3x3(nc, fsh, lambda ti, ih: com_v[:, lvl, ti, ih], om_ps, rows=48)

        om01 = omp.tile([32, RC], F32, tag="om01")
        omv0 = om_ps[0:32, :].rearrange("p (z c) -> p z c", z=2)[:, :, 0:480]
        nc.scalar.activation(om01[:], omv0, AF.Identity,
                             bias=comb_t[0:32, lvl:lvl + 1])
        m16 = omp.tile([16, RC], F32, tag="m16")
        omv1 = om_ps[32:48, :].rearrange("p (z c) -> p z c", z=2)[:, :, 0:480]
        nc.scalar.activation(m16[:], omv1, AF.Sigmoid,
                             bias=comb_t[32:48, lvl:lvl + 1])

        # shuffle into [64,480] (p = yx*32 + rcb*16 + t) / [32,480]
        pos0 = s64p.tile([64, 480], F32, tag="s64")
        for yx in range(2):
            for rcb in range(2):
                nc.sync.dma_start(
                    pos0[yx * 32 + rcb * 16: yx * 32 + rcb * 16 + 16, :],
                    om01[yx * 16:yx * 16 + 16, rcb * 480:(rcb + 1) * 480])
        m32 = s32p.tile([32, 480], F32, tag="s32")
        for rcb in range(2):
            nc.sync.dma_start(m32[rcb * 16:rcb * 16 + 16, :],
                              m16[:, rcb * 480:(rcb + 1) * 480])

        # ---- small math ----
        cnt = [0]

        def t64():
            cnt[0] += 1
            return s64p.tile([64, 480], F32, tag="s64", name=f"t64_{ci}_{cnt[0]}")

        def t32():
            cnt[0] += 1
            return s32p.tile([32, 480], F32, tag="s32", name=f"t32_{ci}_{cnt[0]}")

        # positions already carry the +1 grid shift (baked into byx);
        # valid range is [0, H] / [0, W] in grid coords.
        pos = t64()
        nc.vector.tensor_tensor(pos[:], pos0[:],
                                byx_t[:, lvl * 480:(lvl + 1) * 480], A.add)
        sh = t64()
        nc.vector.tensor_scalar(sh[:], pos[:], 1024.0, None, A.add)
        i32t = i32p.tile([64, 480], I32, tag="i32")
        nc.vector.tensor_copy(i32t[:], sh[:])
        ff = t64()
        nc.vector.tensor_copy(ff[:], i32t[:])
        gt = t64()
        nc.vector.tensor_tensor(gt[:], ff[:], sh[:], A.is_gt)
        fl = t64()
        nc.vector.scalar_tensor_tensor(fl[:], ff[:], -1024.0, gt[:], A.add, A.subtract)
        frac = t64()
        nc.vector.tensor_tensor(frac[:], pos[:], fl[:], A.subtract)
        c0 = t64()
        nc.vector.tensor_scalar(c0[:], fl[:], 0.0, hi0_t[:, lvl:lvl + 1],
                                A.max, A.min)
        V = t64()
        nc.vector.tensor_tensor(V[:], c0[:], fl[:], A.is_equal)
        u = t64()
        nc.vector.tensor_scalar(u[:], frac[:], -1.0, 1.0, A.mult, A.add)

        # mask' = m * Vy * Vx  (x rows copied down to base partition 0)
        vx32 = t32()
        nc.vector.tensor_copy(vx32[:], V[32:64, :])
        mv = t32()
        nc.vector.tensor_tensor(mv[:], m32[:], V[0:32, :], A.mult)
        mm_ = t32()
        nc.vector.tensor_tensor(mm_[:], mv[:], vx32[:], A.mult)
        A0 = t32()
        nc.vector.tensor_tensor(A0[:], u[0:32, :], mm_[:], A.mult)
        A1 = t32()
        nc.vector.tensor_tensor(A1[:], frac[0:32, :], mm_[:], A.mult)
        xs0 = t32()
        nc.vector.tensor_copy(xs0[:], u[32:64, :])
        xs1 = t32()
        nc.vector.tensor_copy(xs1[:], frac[32:64, :])

        # wall [32, (cy, px, 480)] bf16
        wall = walp.tile([32, 4 * 480], BF16, tag="wall")
        nc.vector.tensor_tensor(wall[:, 0 * 480:1 * 480], A0[:], xs0[:], A.mult)
        nc.vector.tensor_tensor(wall[:, 1 * 480:2 * 480], A0[:], xs1[:], A.mult)
        nc.vector.tensor_tensor(wall[:, 2 * 480:3 * 480], A1[:], xs0[:], A.mult)
        nc.vector.tensor_tensor(wall[:, 3 * 480:4 * 480], A1[:], xs1[:], A.mult)

        # gather indices: idx = c0y * (W+1) + c0x  (grid coords)
        psx = t32()
        nc.vector.tensor_copy(psx[:], c0[32:64, :])
        gyt = t32()
        nc.vector.scalar_tensor_tensor(gyt[:], c0[0:32, :], float(Wt),
                                       psx[:], A.mult, A.add)
        i16t = smi.tile([32, 480], I16, tag="i16")
        nc.vector.tensor_copy(i16t[:], gyt[:])
        dflat = drp.tile([32, 480], I16, tag="dfl")
        nc.sync.dma_start(dflat[:], i16t[:])
        # wrap_z[p', t*30+cc] = dflat[(z*16+t)*480 + cc*16 + p']
        dfv = dflat[:].rearrange("p c -> (p c)")
        rep = smi.tile([128, NT * 64], I16, tag="rep")
        nc.vector.memset(rep[:], 0)
        repv = rep[:].rearrange("p (t z cc) -> p t z cc", t=NT, z=2, cc=32)
        for z in range(2):
            wrap = smi.tile([16, NT * 30], I16, tag="wrap",
                            name=f"wrap_{ci}_{z}")
            src = dfv[z * 16 * 480:(z + 1) * 16 * 480]
            src = src.rearrange("(tc p) -> p tc", p=16)
            nc.scalar.dma_start(wrap[:], src)
            wv_ = wrap[:].rearrange("p (t cc) -> p t cc", t=NT)
            nc.scalar.dma_start(repv[0:16, :, z, 0:30], wv_)
        # log-double rows 0:16 -> 128
        for k in (16, 32, 64):
            nc.scalar.dma_start(rep[k:2 * k, :], rep[0:k, :])

        # dc accumulator [2][128, 1024] ((z,512)-chunked, 480 used)
        dcs = [psd.tile([128, 1024], F32, tag=f"dc{oh}", name=f"dc_{ci}_{oh}")
               for oh in range(2)]

        fpv = fp_ap[lvl]

        for t in range(NT):
            # per-tap weight row: wrow [1, (cy,px,z,480)] <- wall rows {t,16+t}
            wrow = wrp.tile([1, 4 * 960], BF16, tag="wrow")
            for z in range(2):
                wsrc = vp(wall[t + 16 * z:t + 16 * z + 1, :],
                          [[1920, 1], [480, 4], [1, 480]])
                wdst = vp(wrow[:], [[3840, 1], [960, 4], [1, 480]],
                          doff=z * 480)
                nc.sync.dma_start(wdst, wsrc)

            # PE broadcast: wallb [128, (cy,px,z,480)] bf16 via PSUM ping-pong
            wallb = wbp.tile([128, 4 * 960], BF16, tag="wallb")
            for j in range(4):
                bc = pso.tile([128, 1024], F32, tag="ps", name=f"bc_{ci}_{t}_{j}")
                for z in range(2):
                    nc.tensor.matmul(bc[:, z * 512:z * 512 + 480],
                                     ones_t[:, 0:128],
                                     wrow[0:1, j * 960 + z * 480:
                                          j * 960 + (z + 1) * 480],
                                     start=True, stop=True)
                bcv = bc[:].rearrange("p (z c) -> p z c", z=2)[:, :, 0:480]
                wbv = wallb[:, j * 960:(j + 1) * 960].rearrange(
                    "p (z c) -> p z c", z=2)
                nc.scalar.activation(wbv, bcv, AF.Copy)

            # gather: one 2KB element per sample = full 2x2 patch
            g = gat.tile([128, 8 * 1024], BF16, tag="g")
            gv = g[:].rearrange("p (j i) -> p j i", j=8)
            nc.gpsimd.dma_gather(gv, fpv, rep[:, t * 64:(t + 1) * 64],
                                 1024, 1024, 1024, transpose=True,
                                 single_packet=False)

            # in-place mul: p = g * wall (per corner row cy)
            gb = g[:]
            for cy in range(2):
                pv = vp(gb, [[8192, 128], [2048, 2], [1024, 2], [512, 2],
                             [1, 480]], doff=cy * 4096)
                wv = vp(wallb[:], [[3840, 128], [960, 2], [0, 2], [480, 2],
                                   [1, 480]], doff=cy * 1920)
                nc.vector.tensor_tensor(pv, pv, wv, A.mult)

            # q = p[cy0] + p[cy1]   [128, (px, hl, z, 480)]
            q = qp.tile([128, 4 * 960], BF16, tag="q")
            qv = vp(q[:], [[3840, 128], [960, 4], [480, 2], [1, 480]])
            p0 = vp(gb, [[8192, 128], [1024, 4], [512, 2], [1, 480]])
            p1 = vp(gb, [[8192, 128], [1024, 4], [512, 2], [1, 480]],
                    doff=4096)
            nc.vector.tensor_tensor(qv, p0, p1, A.add)

            # s = q[px0] + q[px1]   [128, (hl, z, 480)]
            s_ = sp.tile([128, 2 * 960], BF16, tag="s")
            sv = vp(s_[:], [[1920, 128], [960, 2], [480, 2], [1, 480]])
            qa = vp(q[:], [[3840, 128], [960, 2], [480, 2], [1, 480]])
            qb = vp(q[:], [[3840, 128], [960, 2], [480, 2], [1, 480]],
                    doff=1920)
            nc.vector.tensor_tensor(sv, qa, qb, A.add)

            sview = s_[:].rearrange("p (h z c) -> p h z c", h=2, z=2)
            for oh in range(2):
                for ih in range(2):
                    for z in range(2):
                        nc.tensor.matmul(
                            dcs[oh][:, z * 512:z * 512 + 480],
                            dcn_v[:, t, ih, oh],
                            sview[:, ih, z, :],
                            start=(t == 0 and ih == 0),
                            stop=(t == NT - 1 and ih == 1))

        # f update: f += relu(dc + b)
        for h in range(2):
            rel = fup.tile([128, RC], F32, tag="rel")
            dcv = dcs[h][:].rearrange("p (z c) -> p z c", z=2)[:, :, 0:480]
            nc.scalar.activation(rel[:], dcv, AF.Relu,
                                 bias=dcnb_t[:, 2 * lvl + h:2 * lvl + h + 1])
            fiv = fmas[h][:].rearrange("p (r c) -> p r c", c=FW)[:, 1:25, 1:41]
            rv = rel[:].rearrange("p (r c) -> p r c", c=HOUT)
            nc.vector.tensor_tensor(fiv, fiv, rv, A.add)
            fsv = fsh[h][:].rearrange("p (r c) -> p r c", c=FW)[:, 1:25, 1:41]
            nc.vector.tensor_copy(fsv, fiv)

    # ---- residual conv + fh ----------------------------------------------
    res_t = wgt.tile([128, 9 * 2 * 2 * 128], BF16, tag="res")
    nc.sync.dma_start(res_t[:], dt["res_w"])
    res_v = res_t[:].rearrange("p (t i o q) -> p t i o q", t=9, i=2, o=2)
    for oh in range(2):
        rps = psd.tile([128, 1024], F32, tag=f"dc{oh}", name=f"rps_{oh}")
        conv3x3(nc, fsh, lambda ti, ih, oh=oh: res_v[:, ti, ih, oh], rps)
        ot = fup.tile([128, RC], F32, tag="ot")
        rpv = rps[:].rearrange("p (z c) -> p z c", z=2)[:, :, 0:480]
        nc.scalar.activation(ot[:], rpv, AF.Identity, bias=resb_t[:, oh:oh + 1])
        nc.vector.tensor_tensor(ot[:], ot[:], fh_t[:, oh * RC:(oh + 1) * RC], A.add)
        nc.sync.dma_start(out_d[128 * oh:128 * (oh + 1), :], ot[:])


def conv3x3(nc, fsh, w_fn, out_ps, rows=128):
    """3x3 stride-1 conv over the padded f window; out [rows, (z,512|480)]."""
    taps = [(a, b) for a in (-1, 0, 1) for b in (-1, 0, 1)]
    for ti, (dy, dx) in enumerate(taps):
        for ih in range(2):
            rhs = fsh[ih][:].rearrange("p (r c) -> p r c", c=FW)
            for nh in range(2):
                nc.tensor.matmul(
                    out_ps[0:rows, nh * 512:nh * 512 + 480],
                    w_fn(ti, ih),
                    rhs[:, 1 + dy + nh * 12:1 + dy + nh * 12 + 12,
                        1 + dx:1 + dx + 40],
                    start=(ti == 0 and ih == 0), stop=(ti == 8 and ih == 1))


# ===========================================================================
# host side
# ===========================================================================

def packed_table(f):
    """[(H+1)*(W+1), 1024] bf16: entry (yy,xx) = 2x2 patch at (yy-1, xx-1)."""
    Cc, H, W = f.shape
    fpad = np.zeros((Cc, H + 2, W + 2), np.float32)
    fpad[:, 1:H + 1, 1:W + 1] = f
    parts = [fpad[:, dy:dy + H + 1, dx:dx + W + 1]
             for dy, dx in ((0, 0), (0, 1), (1, 0), (1, 1))]
    t = np.stack(parts, axis=0)            # [4, C, H+1, W+1]
    t = t.transpose(2, 3, 0, 1)            # [H+1, W+1, 4, C]
    return np.ascontiguousarray(
        t.reshape((H + 1) * (W + 1), 4 * Cc)).astype(ml_dtypes.bfloat16)


def prep_core_inputs(inputs, b, half):
    """Per-core input map for image b, row-half `half` (0=top)."""
    g0 = 0 if half == 0 else 16
    f0 = np.asarray(inputs["f0"][b], np.float32)
    f1 = np.asarray(inputs["f1"][b], np.float32)
    f2 = np.asarray(inputs["f2"][b], np.float32)

    finit = np.zeros((C, FR, FW), np.float32)
    for r in range(FR):
        gr = g0 - 1 + r
        if 0 <= gr < HOUT:
            finit[:, r, 1:41] = f2[:, gr, :]

    # fh as [128, (oh, rc)]
    fh0 = f2[:, g0:g0 + ROWS, :].reshape(C, RC)
    fh = np.concatenate([fh0[:128], fh0[128:]], axis=1)

    # base positions in +1-shifted grid coords
    byx = np.zeros((2, 64, 480), np.float32)
    hi0 = np.zeros((2, 64, 1), np.float32)
    for lvl in range(2):
        k_, st_, pad_, dil_ = CONFIGS[lvl]
        Hin = HIN[lvl]
        rc = np.arange(480)
        for rcb in range(2):
            rr = (rcb * 480 + rc) // HOUT
            cc = (rcb * 480 + rc) % HOUT
            for t in range(NT):
                byx[lvl, rcb * 16 + t] = st_ * (g0 + rr) - pad_ + (t // k_) * dil_ + 1
                byx[lvl, 32 + rcb * 16 + t] = st_ * cc - pad_ + (t % k_) * dil_ + 1
        hi0[lvl, 0:32] = Hin      # clamp hi in shifted coords
        hi0[lvl, 32:64] = Hin
    byx = byx.transpose(1, 0, 2).reshape(64, 2 * 480)
    hi0 = hi0.transpose(1, 0, 2).reshape(64, 2)

    perm = list(range(0, 32, 2)) + list(range(1, 32, 2)) + list(range(32, 48))
    com_w = np.zeros((2, 9, 2, 128, 48), np.float32)
    com_b = np.zeros((2, 48, 1), np.float32)
    dcn_w = np.zeros((2, NT, 2, 2, 128, 128), np.float32)
    dcn_b = np.zeros((2, 2, 128, 1), np.float32)
    for lvl in range(2):
        cw = np.asarray(inputs[f"com_w{lvl}"], np.float32)[perm]
        cb = np.asarray(inputs[f"com_b{lvl}"], np.float32)[perm]
        for ty in range(3):
            for tx in range(3):
                for ih in range(2):
                    com_w[lvl, ty * 3 + tx, ih] = \
                        cw[:, ih * 128:(ih + 1) * 128, ty, tx].T
        com_b[lvl, :, 0] = cb
        dw = np.asarray(inputs[f"dcn_w{lvl}"], np.float32)
        for k in range(NT):
            for ih in range(2):
                for oh in range(2):
                    dcn_w[lvl, k, ih, oh] = dw[oh * 128:(oh + 1) * 128,
                                               ih * 128:(ih + 1) * 128,
                                               k // 4, k % 4].T
        db = np.asarray(inputs[f"dcn_b{lvl}"], np.float32)
        dcn_b[lvl, 0, :, 0] = db[:128]
        dcn_b[lvl, 1, :, 0] = db[128:]
    rw = np.asarray(inputs["res_w"], np.float32)
    res_w = np.zeros((9, 2, 2, 128, 128), np.float32)
    for ty in range(3):
        for tx in range(3):
            for ih in range(2):
                for oh in range(2):
                    res_w[ty * 3 + tx, ih, oh] = rw[oh * 128:(oh + 1) * 128,
                                                    ih * 128:(ih + 1) * 128,
                                                    ty, tx].T
    rb = np.asarray(inputs["res_b"], np.float32)
    res_b = np.stack([rb[:128], rb[128:]], axis=1)  # [128, 2]

    com_w = com_w.transpose(3, 0, 1, 2, 4).reshape(128, -1)
    com_b = com_b.transpose(1, 0, 2).reshape(48, 2)
    dcn_w = dcn_w.transpose(0, 4, 1, 2, 3, 5).reshape(2, 128, -1)
    dcn_b = dcn_b.transpose(2, 0, 1, 3).reshape(128, 4)
    res_w = res_w.transpose(3, 0, 1, 2, 4).reshape(128, -1)

    return {
        "fp0": packed_table(f0),
        "fp1": packed_table(f1),
        "finit": finit.reshape(C, FSZ),
        "fh": fh.astype(np.float32),
        "byx": byx,
        "hi0": hi0,
        "com_w": com_w.astype(ml_dtypes.bfloat16),
        "com_b": np.ascontiguousarray(com_b),
        "dcn_w": np.ascontiguousarray(dcn_w).astype(ml_dtypes.bfloat16),
        "dcn_b": np.ascontiguousarray(dcn_b),
        "res_w": np.ascontiguousarray(res_w).astype(ml_dtypes.bfloat16),
        "res_b": np.ascontiguousarray(res_b).astype(np.float32),
    }


def assemble_output(results):
    out = np.zeros((B, C, HOUT, HOUT), np.float32)
    for b in range(B):
        top = np.asarray(results[2 * b]["out"]).reshape(C, ROWS, HOUT)
        bot = np.asarray(results[2 * b + 1]["out"]).reshape(C, ROWS, HOUT)
        out[b, :, 0:20, :] = top[:, 0:20, :]
        out[b, :, 20:40, :] = bot[:, 4:24, :]
    return out


_NC_CACHE = []


def kernel(**inputs):
    if not _NC_CACHE:
        _NC_CACHE.append(build_program())
    nc = _NC_CACHE[0]
    in_maps = [prep_core_inputs(inputs, b, half)
               for b in range(B) for half in range(2)]
    from concourse.bass_utils import run_bass_kernel_spmd
    r = run_bass_kernel_spmd(nc, in_maps, list(range(8)))
    return assemble_output(r.results)


# revision 86
# speedup vs baseline: 1.3865x; 1.2886x over previous
"""DCN-FPN Trainium2 kernel (nn_DCNFPN), v2.

Sharding: 8 cores = 4 images x 2 row-halves. Each core computes rows
[g0, g0+23] of every 40-row intermediate (g0 = 0 top / 16 bottom), with
shrinking-validity redundancy so no cross-core communication is needed;
host keeps rows 0..19 (top) / 20..39 (bottom) of the output.

Key structure (vs v1): the DRAM feature table packs the full 2x2
bilinear patch per entry -- entry (yy, xx) of an (H+1)x(W+1) grid holds
[f[yy-1,xx-1], f[yy-1,xx], f[yy,xx-1], f[yy,xx]] over 256 channels
(zero-filled out of bounds), 2 KB each.  One dma_gather per tap fetches
all four corners; OOB x/y handling collapses into table zeros plus one
per-axis clamp-indicator folded into the mask.  The four slot weights
(A0,A1)x(xs0,xs1) are broadcast to 128 partitions through the PE (ones
[1,128] matmul) and copied PSUM->SBUF bf16 by the Activation engine --
no DRAM round trip.  Corner combine: 2 in-place TT muls + q/s adds on
DVE (bf16, 2x mode), then 8 PSUM-accumulating matmuls per tap.

Per call: offset conv (36 mm) -> om activations -> shuffle -> small
math ([64,480]: floor/frac/clamp/valid; walls+idx on [32,480]) -> idx
i16 wrap via DRAM -> 16-tap pipeline -> f += relu(dc).
Final: residual conv + fh, store [256, 960] fp32.

Sample enumeration per tap: gather column i = 512*z + 16*cc + p
(z = rc//480, p = rc%16, cc = (rc%480)//16); columns 480:512 of each
512-block are pad (idx 0, ignored).
"""
import sys
sys.path.insert(0, "/opt/trn_rl_repo")

from contextlib import ExitStack
import numpy as np
import ml_dtypes

import bass_rust
import concourse.bass as bass
import concourse.bacc as bacc
import concourse.mybir as mybir
import concourse.tile as tile

F32 = mybir.dt.float32
BF16 = mybir.dt.bfloat16
I16 = mybir.dt.int16
I32 = mybir.dt.int32
A = mybir.AluOpType
AF = mybir.ActivationFunctionType

B, C, HOUT = 4, 256, 40
CONFIGS = [(4, 2, 1, 1), (4, 4, 3, 3)]   # (k, stride, pad, dil)
HIN = [80, 160]                          # per level l=0 (f1), l=1 (f0)
TW = [HIN[0] + 1, HIN[1] + 1]            # packed-table grid width per level
ROWS = 24                                # out rows per core per call
RC = ROWS * HOUT                         # 960
NT = 16                                  # taps
CALLS = [0, 1, 0, 1]
FW = 42                                  # padded f width
FR = 26                                  # f window rows
FSZ = FR * FW                            # 1092


def vp(ap, dims, doff=0):
    v = ap.copy()
    v.ap = bass_rust.VecI64Pair(dims)
    if doff:
        v.offset = v.offset + doff
    return v


def build_program():
    nc = bacc.Bacc("TRN2", target_bir_lowering=False, debug=False)

    dt = {}

    def din(name, shape, dtype=F32):
        dt[name] = nc.dram_tensor(name, shape, dtype, kind="ExternalInput").ap()

    din("fp0", [TW[1] * TW[1], 1024], BF16)   # level 1 packed table (f0)
    din("fp1", [TW[0] * TW[0], 1024], BF16)   # level 0 packed table (f1)
    din("finit", [C, FSZ], BF16)
    din("fh", [128, 2 * RC], F32)
    din("byx", [64, 2 * 480], F32)
    din("hi0", [64, 2], F32)
    din("sel", [32, 32 * 128], BF16)
    din("pperm", [32, 4 * 32], F32)
    din("mperm", [16, 2 * 32], F32)
    din("ident", [128, 128], F32)
    din("com_w", [128, 2 * 9 * 2 * 48], BF16)
    din("com_b", [48, 2], F32)
    din("dcn_w", [2, 128, NT * 2 * 2 * 128], BF16)
    din("dcn_b", [128, 4], F32)
    din("res_w", [128, 9 * 2 * 2 * 128], BF16)
    din("res_b", [128, 2], F32)
    out_d = nc.dram_tensor("out", [C, RC], F32, kind="ExternalOutput").ap()

    with tile.TileContext(nc) as tc, ExitStack() as ctx:
        build_body(nc, tc, ctx, dt, out_d)
    nc.compile()
    return nc


def build_body(nc, tc, ctx, dt, out_d):
    cst = ctx.enter_context(tc.tile_pool(name="cst", bufs=1))
    s64p = ctx.enter_context(tc.tile_pool(name="s64p", bufs=4))
    s32p = ctx.enter_context(tc.tile_pool(name="s32p", bufs=4))
    smi = ctx.enter_context(tc.tile_pool(name="smi", bufs=1))
    omp = ctx.enter_context(tc.tile_pool(name="omp", bufs=1))
    wgt = ctx.enter_context(tc.tile_pool(name="wgt", bufs=1))
    walp = ctx.enter_context(tc.tile_pool(name="walp", bufs=1))
    wbp = ctx.enter_context(tc.tile_pool(name="wbp", bufs=3))
    gat = ctx.enter_context(tc.tile_pool(name="gat", bufs=6))
    qp = ctx.enter_context(tc.tile_pool(name="qp", bufs=4))
    fup = ctx.enter_context(tc.tile_pool(name="fup", bufs=2))
    pso = ctx.enter_context(tc.tile_pool(name="pso", bufs=2, space="PSUM"))
    psd = ctx.enter_context(tc.tile_pool(name="psd", bufs=1, space="PSUM"))
    drp = ctx.enter_context(tc.tile_pool(name="drp", bufs=2, space="DRAM"))

    # ---- persistent loads (critical first; spread across SP/Act queues) --
    com_t = cst.tile([128, 2 * 9 * 2 * 48], BF16, tag="com")
    nc.sync.dma_start(com_t[:], dt["com_w"])
    com_v = com_t[:].rearrange("p (l t i o) -> p l t i o", l=2, t=9, i=2, o=48)

    fsh = []
    for h in range(2):
        fs = cst.tile([128, FSZ], BF16, tag=f"fsh{h}")
        nc.sync.dma_start(fs[:], dt["finit"][128 * h:128 * (h + 1), :])
        fsh.append(fs)

    byx_t = cst.tile([64, 2 * 480], F32, tag="byx")
    nc.scalar.dma_start(byx_t[:], dt["byx"])
    hi0_t = cst.tile([64, 2], F32, tag="hi0")
    nc.scalar.dma_start(hi0_t[:], dt["hi0"])
    comb_t = cst.tile([48, 2], F32, tag="comb")
    nc.scalar.dma_start(comb_t[:], dt["com_b"])
    sel_t = cst.tile([32, 32 * 128], BF16, tag="sel")
    nc.scalar.dma_start(sel_t[:], dt["sel"])
    sel_v = sel_t[:].rearrange("p (r o) -> p r o", r=32)
    pperm_t = cst.tile([32, 4 * 32], F32, tag="pperm")
    nc.scalar.dma_start(pperm_t[:], dt["pperm"])
    pperm_v = pperm_t[:].rearrange("p (v o) -> p v o", v=4)
    mperm_t = cst.tile([16, 2 * 32], F32, tag="mperm")
    nc.scalar.dma_start(mperm_t[:], dt["mperm"])
    mperm_v = mperm_t[:].rearrange("p (v o) -> p v o", v=2)
    ident_t = cst.tile([128, 128], F32, tag="ident")
    nc.scalar.dma_start(ident_t[:], dt["ident"])
    dcnb_t = cst.tile([128, 4], F32, tag="dcnb")
    nc.scalar.dma_start(dcnb_t[:], dt["dcn_b"])
    resb_t = cst.tile([128, 2], F32, tag="resb")
    nc.scalar.dma_start(resb_t[:], dt["res_b"])
    fh_t = cst.tile([128, 2 * RC], F32, tag="fh")
    nc.scalar.dma_start(fh_t[:], dt["fh"])

    # per-level DCN weights, loaded once
    dcn_ts = []
    for lvl in range(2):
        t_ = cst.tile([128, NT * 2 * 2 * 128], BF16, tag=f"dcn{lvl}")
        nc.scalar.dma_start(t_[:], dt["dcn_w"][lvl])
        dcn_ts.append(t_[:].rearrange("p (k i o q) -> p k i o q",
                                      k=NT, i=2, o=2, q=128))

    fp_ap = {0: dt["fp1"], 1: dt["fp0"]}

    # DRAM staging tile for wrapped gather indices; zero it once so the
    # per-(t,z) pad lanes (cc 30:32) read as index 0 in every call.
    repD = drp.tile([16, NT * 64], I16, tag="repD")
    zs16 = smi.tile([16, NT * 64], I16, tag="zs16")
    nc.vector.memset(zs16[:], 0)
    nc.sync.dma_start(repD[:], zs16[:])

    # ---- per-call ---------------------------------------------------------
    for ci, lvl in enumerate(CALLS):
        Win = HIN[lvl]
        Wt = TW[lvl]
        dcn_v = dcn_ts[lvl]

        # offset conv: om_ps rows 0:48, (z,512)-chunked, 480 used
        om_ps = pso.tile([128, 1024], F32, tag="ps", name=f"omps_{ci}")
        conv3x3(nc, fsh, lambda ti, ih: com_v[:, lvl, ti, ih], om_ps, rows=48)

        # mask activation (com_b offset-bias is folded into byx host-side)
        m16 = omp.tile([16, RC], F32, tag="m16")
        omv1 = om_ps[32:48, :].rearrange("p (z c) -> p z c", z=2)[:, :, 0:480]
        nc.scalar.activation(m16[:], omv1, AF.Sigmoid,
                             bias=comb_t[32:48, lvl:lvl + 1])

        # stage offsets PSUM->SBUF, then shuffle into [64,480]
        # (p = yx*32 + rcb*16 + t) / [32,480] via PE permutation matmuls
        om01 = omp.tile([32, RC], F32, tag="om01")
        omv0 = om_ps[0:32, :].rearrange("p (z c) -> p z c", z=2)[:, :, 0:480]
        nc.scalar.activation(om01[:], omv0, AF.Copy)
        pos0ps = pso.tile([128, 1024], F32, tag="ps", name=f"pos0ps_{ci}")
        for yx in range(2):
            for rcb in range(2):
                nc.tensor.matmul(
                    pos0ps[yx * 32:(yx + 1) * 32, 0:480],
                    pperm_v[:, yx * 2 + rcb, :],
                    om01[0:32, rcb * 480:(rcb + 1) * 480],
                    start=(rcb == 0), stop=(rcb == 1))
        pos0 = pos0ps[0:64, 0:480]
        m32ps = pso.tile([128, 1024], F32, tag="ps", name=f"m32ps_{ci}")
        for rcb in range(2):
            nc.tensor.matmul(m32ps[0:32, 0:480], mperm_v[:, rcb, :],
                             m16[:, rcb * 480:(rcb + 1) * 480],
                             start=(rcb == 0), stop=(rcb == 1))
        m32 = m32ps[0:32, 0:480]

        # ---- small math ----
        cnt = [0]

        def t64():
            cnt[0] += 1
            return s64p.tile([64, 480], F32, tag="s64", name=f"t64_{ci}_{cnt[0]}")

        def t32():
            cnt[0] += 1
            return s32p.tile([32, 480], F32, tag="s32", name=f"t32_{ci}_{cnt[0]}")

        def t64i():
            cnt[0] += 1
            return s64p.tile([64, 480], I32, tag="s64i", bufs=1,
                             name=f"t64i_{ci}_{cnt[0]}")

        # positions carry a +1+1024 shift (baked into byx): +1 for the grid,
        # +1024 so floor-via-mod sees positive operands on hardware.
        # --- idx-critical path first (high priority: gathers wait on it) ---
        hp = tc.high_priority()
        hp.__enter__()
        sh = t64()
        nc.vector.tensor_tensor(sh[:], pos0,
                                byx_t[:, lvl * 480:(lvl + 1) * 480], A.add)
        i32t = t64i()
        nc.vector.tensor_copy(i32t[:], sh[:])
        ff = t64()
        nc.vector.tensor_copy(ff[:], i32t[:])
        gt = t64()
        nc.vector.tensor_tensor(gt[:], ff[:], sh[:], A.is_gt)
        fls = t64()
        nc.vector.tensor_tensor(fls[:], ff[:], gt[:], A.subtract)
        c0 = t64()
        nc.vector.tensor_scalar(c0[:], fls[:], 1024.0, hi0_t[:, lvl:lvl + 1],
                                A.max, A.min)
        # gather idx = (c0y-1024)*Wt + c0x-1024  (psx pre-subtracts the shift)
        psx = t32()
        nc.vector.tensor_scalar(psx[:], c0[32:64, :],
                                -1024.0 * (Wt + 1.0), None, A.add)
        gyt = t32()
        nc.vector.scalar_tensor_tensor(gyt[:], c0[0:32, :], float(Wt),
                                       psx[:], A.mult, A.add)
        i16t = smi.tile([32, 480], I16, tag="i16")
        nc.vector.tensor_copy(i16t[:], gyt[:])
        dflat = drp.tile([32, 480], I16, tag="dfl")
        nc.sync.dma_start(dflat[:], i16t[:])

        # idx wrap via DRAM: repD[p', t*64+z*32+cc] = dflat[(z*16+t)*480
        # + cc*16 + p'] (DRAM->DRAM strided, chunked by tap-half x z on two
        # queues), then one broadcast DMA fills all 8 replica row-groups.
        # repD pad lanes (cc 30:32) are zeroed once at kernel start.
        dfv = dflat[:].rearrange("p c -> (p c)")
        rdv = repD[:].rearrange("p (t z cc) -> p t z cc", t=NT, z=2, cc=32)
        HT = NT // 2
        for th, eng in ((0, nc.sync), (1, nc.scalar)):
            for z in range(2):
                wrap = smi.tile([16, HT * 30], I16, tag=f"wrap{th}{z}",
                                name=f"wrap_{ci}_{th}_{z}")
                base = (z * 16 + th * HT) * 480
                src = dfv[base:base + HT * 480]
                src = src.rearrange("(tc p) -> p tc", p=16)
                eng.dma_start(wrap[:], src)
                wv_ = wrap[:].rearrange("p (t cc) -> p t cc", t=HT)
                eng.dma_start(rdv[0:16, th * HT:(th + 1) * HT, z, 0:30], wv_)
        rep = smi.tile([128, NT * 64], I16, tag="rep")
        for grp in range(8):
            eng = nc.sync if grp % 2 == 0 else nc.scalar
            eng.dma_start(rep[grp * 16:(grp + 1) * 16, :], repD[:])
        hp.__exit__(None, None, None)

        # --- weight path (overlaps the idx DMA chain) ---
        frac = t64()
        nc.vector.tensor_tensor(frac[:], sh[:], fls[:], A.subtract)
        V = t64()
        nc.vector.tensor_tensor(V[:], c0[:], fls[:], A.is_equal)
        u = t64()
        nc.vector.tensor_scalar(u[:], frac[:], -1.0, 1.0, A.mult, A.add)

        # mask' = m * Vy * Vx  (x rows copied down to base partition 0;
        # weight-path copies on Act, off the DVE critical path)
        vx32 = t32()
        nc.scalar.copy(vx32[:], V[32:64, :])
        mv = t32()
        nc.vector.tensor_tensor(mv[:], m32, V[0:32, :], A.mult)
        mm_ = t32()
        nc.vector.tensor_tensor(mm_[:], mv[:], vx32[:], A.mult)
        A0 = t32()
        nc.vector.tensor_tensor(A0[:], u[0:32, :], mm_[:], A.mult)
        A1 = t32()
        nc.vector.tensor_tensor(A1[:], frac[0:32, :], mm_[:], A.mult)
        xs0 = t32()
        nc.scalar.copy(xs0[:], u[32:64, :])
        xs1 = t32()
        nc.scalar.copy(xs1[:], frac[32:64, :])

        # wall [32, (cy, px, 480)] bf16
        wall = walp.tile([32, 4 * 480], BF16, tag="wall")
        nc.vector.tensor_tensor(wall[:, 0 * 480:1 * 480], A0[:], xs0[:], A.mult)
        nc.vector.tensor_tensor(wall[:, 1 * 480:2 * 480], A0[:], xs1[:], A.mult)
        nc.vector.tensor_tensor(wall[:, 2 * 480:3 * 480], A1[:], xs0[:], A.mult)
        nc.vector.tensor_tensor(wall[:, 3 * 480:4 * 480], A1[:], xs1[:], A.mult)

        # dc accumulator [2][128, 1024] ((z,512)-chunked, 480 used)
        dcs = [psd.tile([128, 1024], F32, tag=f"dc{oh}", name=f"dc_{ci}_{oh}")
               for oh in range(2)]

        fpv = fp_ap[lvl]

        def emit_bcast(t):
            # PE broadcast via one-hot selector: bc[o,c] = wall[t+16z, c]
            wallb = wbp.tile([128, 4 * 960], BF16, tag="wallb",
                             name=f"wallb_{ci}_{t}")
            for j in range(4):
                bc = pso.tile([128, 1024], F32, tag="ps", name=f"bc_{ci}_{t}_{j}")
                for z in range(2):
                    nc.tensor.matmul(bc[:, z * 512:z * 512 + 480],
                                     sel_v[:, t + 16 * z, :],
                                     wall[0:32, j * 480:(j + 1) * 480],
                                     start=True, stop=True)
                bcv = bc[:].rearrange("p (z c) -> p z c", z=2)[:, :, 0:480]
                wbv = wallb[:, j * 960:(j + 1) * 960].rearrange(
                    "p (z c) -> p z c", z=2)
                nc.scalar.activation(wbv, bcv, AF.Copy)
            return wallb

        def emit_gather(t, z):
            # gather: one 2KB element per sample = full 2x2 patch; half-tap
            g = gat.tile([128, 8 * 512], BF16, tag="g", name=f"g_{ci}_{t}_{z}")
            gv = g[:].rearrange("p (j i) -> p j i", j=8)
            nc.gpsimd.dma_gather(gv, fpv,
                                 rep[:, t * 64 + z * 32:t * 64 + z * 32 + 32],
                                 512, 512, 1024, transpose=True,
                                 single_packet=False)
            return g

        # all gather dispatches up-front: Pool's in-order queue paces them
        # purely by gat-buffer WAR, never behind a compute op
        gs = {(t, z): emit_gather(t, z) for t in range(NT) for z in range(2)}
        wallbs = {0: emit_bcast(0)}
        for t in range(NT):
            if t + 1 < NT:
                wallbs[t + 1] = emit_bcast(t + 1)
            wallb = wallbs.pop(t)
            for z in range(2):
                g = gs.pop((t, z))
                gb = g[:]

                # in-place mul: p = g * wall  (one op, both corners)
                pv = vp(gb, [[4096, 128], [2048, 2], [1024, 2], [512, 2],
                             [1, 480]])
                wv = vp(wallb[:], [[3840, 128], [1920, 2], [960, 2], [0, 2],
                                   [1, 480]], doff=z * 480)
                nc.vector.tensor_tensor(pv, pv, wv, A.mult)

                # q = p[cy0] + p[cy1]   [128, (px, hl, 480)]
                # z0 on DVE, z1 on Pool
                q = qp.tile([128, 2 * 960], BF16, tag="q",
                            name=f"q_{ci}_{t}_{z}")
                qv = vp(q[:], [[1920, 128], [960, 2], [480, 2], [1, 480]])
                pa = vp(gb, [[4096, 128], [1024, 2], [512, 2], [1, 480]])
                pb = vp(gb, [[4096, 128], [1024, 2], [512, 2], [1, 480]],
                        doff=2048)
                nc.vector.tensor_tensor(qv, pa, pb, A.add)

                # s-sum folded into the matmuls: feed both px halves of q
                qview = q[:].rearrange("p (x h c) -> p x h c", x=2, h=2)
                for oh in range(2):
                    for ih in range(2):
                        for px in range(2):
                            nc.tensor.matmul(
                                dcs[oh][:, z * 512:z * 512 + 480],
                                dcn_v[:, t, ih, oh],
                                qview[:, px, ih, :],
                                start=(t == 0 and ih == 0 and px == 0),
                                stop=(t == NT - 1 and ih == 1 and px == 1))

        # f update: f += relu(dc + b)   (bf16 master; h1 add on Pool so the
        # two halves update in parallel and the conv starts sooner)
        for h in range(2):
            rel = fup.tile([128, RC], BF16, tag="rel", name=f"rel_{ci}_{h}")
            dcv = dcs[h][:].rearrange("p (z c) -> p z c", z=2)[:, :, 0:480]
            nc.scalar.activation(rel[:], dcv, AF.Relu,
                                 bias=dcnb_t[:, 2 * lvl + h:2 * lvl + h + 1])
            fsv = fsh[h][:].rearrange("p (r c) -> p r c", c=FW)[:, 1:25, 1:41]
            rv = rel[:].rearrange("p (r c) -> p r c", c=HOUT)
            (nc.vector if h == 0 else nc.gpsimd).tensor_tensor(
                fsv, fsv, rv, A.add)

    # ---- residual conv + fh ----------------------------------------------
    # fh is pre-accumulated into the PSUM via an identity matmul (start),
    # then the conv taps accumulate on top; output = act(psum + bias).
    res_t = wgt.tile([128, 9 * 2 * 2 * 128], BF16, tag="res")
    nc.sync.dma_start(res_t[:], dt["res_w"])
    res_v = res_t[:].rearrange("p (t i o q) -> p t i o q", t=9, i=2, o=2)
    for oh in range(2):
        rps = psd.tile([128, 1024], F32, tag=f"dc{oh}", name=f"rps_{oh}")
        fhv = fh_t[:].rearrange("p (o z c) -> p o z c", o=2, z=2)
        for z in range(2):
            nc.tensor.matmul(rps[:, z * 512:z * 512 + 480], ident_t[:],
                             fhv[:, oh, z, :], start=True, stop=False)
        conv3x3(nc, fsh, lambda ti, ih, oh=oh: res_v[:, ti, ih, oh], rps,
                accum=True)
        ot = fup.tile([128, RC], F32, tag="ot")
        rpv = rps[:].rearrange("p (z c) -> p z c", z=2)[:, :, 0:480]
        nc.scalar.activation(ot[:], rpv, AF.Identity, bias=resb_t[:, oh:oh + 1])
        nc.sync.dma_start(out_d[128 * oh:128 * (oh + 1), :], ot[:])


def conv3x3(nc, fsh, w_fn, out_ps, rows=128, accum=False):
    """3x3 stride-1 conv over the padded f window; out [rows, (z,512|480)].

    ih-outer so the ih=0 matmuls can start before fsh[1] is updated."""
    taps = [(a, b) for a in (-1, 0, 1) for b in (-1, 0, 1)]
    for ih in range(2):
        rhs = fsh[ih][:].rearrange("p (r c) -> p r c", c=FW)
        for ti, (dy, dx) in enumerate(taps):
            for nh in range(2):
                nc.tensor.matmul(
                    out_ps[0:rows, nh * 512:nh * 512 + 480],
                    w_fn(ti, ih),
                    rhs[:, 1 + dy + nh * 12:1 + dy + nh * 12 + 12,
                        1 + dx:1 + dx + 40],
                    start=(not accum and ih == 0 and ti == 0),
                    stop=(ih == 1 and ti == 8))


# ===========================================================================
# host side
# ===========================================================================

def packed_table(f):
    """[(H+1)*(W+1), 1024] bf16: entry (yy,xx) = 2x2 patch at (yy-1, xx-1)."""
    Cc, H, W = f.shape
    fpad = np.zeros((Cc, H + 2, W + 2), np.float32)
    fpad[:, 1:H + 1, 1:W + 1] = f
    parts = [fpad[:, dy:dy + H + 1, dx:dx + W + 1]
             for dy, dx in ((0, 0), (0, 1), (1, 0), (1, 1))]
    t = np.stack(parts, axis=0)            # [4, C, H+1, W+1]
    t = t.transpose(2, 3, 0, 1)            # [H+1, W+1, 4, C]
    return np.ascontiguousarray(
        t.reshape((H + 1) * (W + 1), 4 * Cc)).astype(ml_dtypes.bfloat16)


def prep_core_inputs(inputs, b, half):
    """Per-core input map for image b, row-half `half` (0=top)."""
    g0 = 0 if half == 0 else 16
    f0 = np.asarray(inputs["f0"][b], np.float32)
    f1 = np.asarray(inputs["f1"][b], np.float32)
    f2 = np.asarray(inputs["f2"][b], np.float32)

    finit = np.zeros((C, FR, FW), np.float32)
    for r in range(FR):
        gr = g0 - 1 + r
        if 0 <= gr < HOUT:
            finit[:, r, 1:41] = f2[:, gr, :]

    # fh as [128, (oh, rc)]
    fh0 = f2[:, g0:g0 + ROWS, :].reshape(C, RC)
    fh = np.concatenate([fh0[:128], fh0[128:]], axis=1)

    perm = list(range(0, 32, 2)) + list(range(1, 32, 2)) + list(range(32, 48))

    # base positions in +1-shifted grid coords; offset-conv bias folded in
    byx = np.zeros((2, 64, 480), np.float32)
    hi0 = np.zeros((2, 64, 1), np.float32)
    for lvl in range(2):
        k_, st_, pad_, dil_ = CONFIGS[lvl]
        Hin = HIN[lvl]
        cbp = np.asarray(inputs[f"com_b{lvl}"], np.float32)[perm]
        rc = np.arange(480)
        for rcb in range(2):
            rr = (rcb * 480 + rc) // HOUT
            cc = (rcb * 480 + rc) % HOUT
            for t in range(NT):
                byx[lvl, rcb * 16 + t] = (st_ * (g0 + rr) - pad_
                                          + (t // k_) * dil_ + 1025 + cbp[t])
                byx[lvl, 32 + rcb * 16 + t] = (st_ * cc - pad_ + (t % k_) * dil_
                                               + 1025 + cbp[16 + t])
        hi0[lvl, 0:32] = 1024 + Hin   # clamp hi in shifted coords
        hi0[lvl, 32:64] = 1024 + Hin
    byx = byx.transpose(1, 0, 2).reshape(64, 2 * 480)
    hi0 = hi0.transpose(1, 0, 2).reshape(64, 2)
    com_w = np.zeros((2, 9, 2, 128, 48), np.float32)
    com_b = np.zeros((2, 48, 1), np.float32)
    dcn_w = np.zeros((2, NT, 2, 2, 128, 128), np.float32)
    dcn_b = np.zeros((2, 2, 128, 1), np.float32)
    for lvl in range(2):
        cw = np.asarray(inputs[f"com_w{lvl}"], np.float32)[perm]
        cb = np.asarray(inputs[f"com_b{lvl}"], np.float32)[perm]
        for ty in range(3):
            for tx in range(3):
                for ih in range(2):
                    com_w[lvl, ty * 3 + tx, ih] = \
                        cw[:, ih * 128:(ih + 1) * 128, ty, tx].T
        com_b[lvl, :, 0] = cb
        dw = np.asarray(inputs[f"dcn_w{lvl}"], np.float32)
        for k in range(NT):
            for ih in range(2):
                for oh in range(2):
                    dcn_w[lvl, k, ih, oh] = dw[oh * 128:(oh + 1) * 128,
                                               ih * 128:(ih + 1) * 128,
                                               k // 4, k % 4].T
        db = np.asarray(inputs[f"dcn_b{lvl}"], np.float32)
        dcn_b[lvl, 0, :, 0] = db[:128]
        dcn_b[lvl, 1, :, 0] = db[128:]
    rw = np.asarray(inputs["res_w"], np.float32)
    res_w = np.zeros((9, 2, 2, 128, 128), np.float32)
    for ty in range(3):
        for tx in range(3):
            for ih in range(2):
                for oh in range(2):
                    res_w[ty * 3 + tx, ih, oh] = rw[oh * 128:(oh + 1) * 128,
                                                    ih * 128:(ih + 1) * 128,
                                                    ty, tx].T
    rb = np.asarray(inputs["res_b"], np.float32)
    res_b = np.stack([rb[:128], rb[128:]], axis=1)  # [128, 2]

    # PE permutation matrices for the om -> pos0 / m16 -> m32 shuffles
    pperm = np.zeros((32, 4, 32), np.float32)
    for yx in range(2):
        for rcb in range(2):
            for t in range(16):
                pperm[yx * 16 + t, yx * 2 + rcb, rcb * 16 + t] = 1.0
    mperm = np.zeros((16, 2, 32), np.float32)
    for rcb in range(2):
        for t in range(16):
            mperm[t, rcb, rcb * 16 + t] = 1.0

    com_w = com_w.transpose(3, 0, 1, 2, 4).reshape(128, -1)
    com_b = com_b.transpose(1, 0, 2).reshape(48, 2)
    dcn_w = dcn_w.transpose(0, 4, 1, 2, 3, 5).reshape(2, 128, -1)
    dcn_b = dcn_b.transpose(2, 0, 1, 3).reshape(128, 4)
    res_w = res_w.transpose(3, 0, 1, 2, 4).reshape(128, -1)

    return {
        "fp0": packed_table(f0),
        "fp1": packed_table(f1),
        "finit": finit.reshape(C, FSZ).astype(ml_dtypes.bfloat16),
        "fh": fh.astype(np.float32),
        "byx": byx,
        "hi0": hi0,
        "sel": np.ascontiguousarray(
            np.tile(np.eye(32, dtype=np.float32)[:, :, None],
                    (1, 1, 128)).reshape(32, 32 * 128)
        ).astype(ml_dtypes.bfloat16),
        "pperm": pperm.reshape(32, 4 * 32),
        "mperm": mperm.reshape(16, 2 * 32),
        "ident": np.eye(128, dtype=np.float32),
        "com_w": com_w.astype(ml_dtypes.bfloat16),
        "com_b": np.ascontiguousarray(com_b),
        "dcn_w": np.ascontiguousarray(dcn_w).astype(ml_dtypes.bfloat16),
        "dcn_b": np.ascontiguousarray(dcn_b),
        "res_w": np.ascontiguousarray(res_w).astype(ml_dtypes.bfloat16),
        "res_b": np.ascontiguousarray(res_b).astype(np.float32),
    }


def assemble_output(results):
    out = np.zeros((B, C, HOUT, HOUT), np.float32)
    for b in range(B):
        top = np.asarray(results[2 * b]["out"]).reshape(C, ROWS, HOUT)
        bot = np.asarray(results[2 * b + 1]["out"]).reshape(C, ROWS, HOUT)
        out[b, :, 0:20, :] = top[:, 0:20, :]
        out[b, :, 20:40, :] = bot[:, 4:24, :]
    return out


_NC_CACHE = []


def kernel(**inputs):
    if not _NC_CACHE:
        _NC_CACHE.append(build_program())
    nc = _NC_CACHE[0]
    in_maps = [prep_core_inputs(inputs, b, half)
               for b in range(B) for half in range(2)]
    from concourse.bass_utils import run_bass_kernel_spmd
    r = run_bass_kernel_spmd(nc, in_maps, list(range(8)))
    return assemble_output(r.results)


# revision 94
# speedup vs baseline: 1.3911x; 1.0033x over previous
"""DCN-FPN Trainium2 kernel (nn_DCNFPN), v2.

Sharding: 8 cores = 4 images x 2 row-halves. Each core computes rows
[g0, g0+23] of every 40-row intermediate (g0 = 0 top / 16 bottom), with
shrinking-validity redundancy so no cross-core communication is needed;
host keeps rows 0..19 (top) / 20..39 (bottom) of the output.

Key structure (vs v1): the DRAM feature table packs the full 2x2
bilinear patch per entry -- entry (yy, xx) of an (H+1)x(W+1) grid holds
[f[yy-1,xx-1], f[yy-1,xx], f[yy,xx-1], f[yy,xx]] over 256 channels
(zero-filled out of bounds), 2 KB each.  One dma_gather per tap fetches
all four corners; OOB x/y handling collapses into table zeros plus one
per-axis clamp-indicator folded into the mask.  The four slot weights
(A0,A1)x(xs0,xs1) are broadcast to 128 partitions through the PE (ones
[1,128] matmul) and copied PSUM->SBUF bf16 by the Activation engine --
no DRAM round trip.  Corner combine: 2 in-place TT muls + q/s adds on
DVE (bf16, 2x mode), then 8 PSUM-accumulating matmuls per tap.

Per call: offset conv (36 mm) -> om activations -> shuffle -> small
math ([64,480]: floor/frac/clamp/valid; walls+idx on [32,480]) -> idx
i16 wrap via DRAM -> 16-tap pipeline -> f += relu(dc).
Final: residual conv + fh, store [256, 960] fp32.

Sample enumeration per tap: gather column i = 512*z + 16*cc + p
(z = rc//480, p = rc%16, cc = (rc%480)//16); columns 480:512 of each
512-block are pad (idx 0, ignored).
"""
import sys
sys.path.insert(0, "/opt/trn_rl_repo")

from contextlib import ExitStack
import numpy as np
import ml_dtypes

import bass_rust
import concourse.bass as bass
import concourse.bacc as bacc
import concourse.mybir as mybir
import concourse.tile as tile

F32 = mybir.dt.float32
BF16 = mybir.dt.bfloat16
I16 = mybir.dt.int16
I32 = mybir.dt.int32
A = mybir.AluOpType
AF = mybir.ActivationFunctionType

B, C, HOUT = 4, 256, 40
CONFIGS = [(4, 2, 1, 1), (4, 4, 3, 3)]   # (k, stride, pad, dil)
HIN = [80, 160]                          # per level l=0 (f1), l=1 (f0)
TW = [HIN[0] + 1, HIN[1] + 1]            # packed-table grid width per level
ROWS = 24                                # out rows per core per call
RC = ROWS * HOUT                         # 960
NT = 16                                  # taps
CALLS = [0, 1, 0, 1]
FW = 42                                  # padded f width
FR = 26                                  # f window rows
FSZ = FR * FW                            # 1092


def vp(ap, dims, doff=0):
    v = ap.copy()
    v.ap = bass_rust.VecI64Pair(dims)
    if doff:
        v.offset = v.offset + doff
    return v


def build_program():
    nc = bacc.Bacc("TRN2", target_bir_lowering=False, debug=False)

    dt = {}

    def din(name, shape, dtype=F32):
        dt[name] = nc.dram_tensor(name, shape, dtype, kind="ExternalInput").ap()

    din("fp0", [TW[1] * TW[1], 1024], BF16)   # level 1 packed table (f0)
    din("fp1", [TW[0] * TW[0], 1024], BF16)   # level 0 packed table (f1)
    din("finit", [C, FSZ], BF16)
    din("fh", [128, 2 * RC], BF16)
    din("byx", [64, 2 * 480], F32)
    din("hi0", [64, 2], F32)
    din("sel", [32, 32 * 128], BF16)
    din("pperm", [32, 4 * 32], F32)
    din("mperm", [16, 2 * 32], F32)
    din("ident", [128, 128], BF16)
    din("com_w", [128, 2 * 9 * 2 * 48], BF16)
    din("com_b", [48, 2], F32)
    din("dcn_w", [2, 128, NT * 2 * 2 * 128], BF16)
    din("dcn_b", [128, 4], F32)
    din("res_w", [128, 9 * 2 * 2 * 128], BF16)
    din("res_b", [128, 2], F32)
    out_d = nc.dram_tensor("out", [C, RC], F32, kind="ExternalOutput").ap()

    with tile.TileContext(nc) as tc, ExitStack() as ctx:
        build_body(nc, tc, ctx, dt, out_d)
    nc.compile()
    return nc


def build_body(nc, tc, ctx, dt, out_d):
    cst = ctx.enter_context(tc.tile_pool(name="cst", bufs=1))
    s64p = ctx.enter_context(tc.tile_pool(name="s64p", bufs=4))
    s32p = ctx.enter_context(tc.tile_pool(name="s32p", bufs=4))
    smi = ctx.enter_context(tc.tile_pool(name="smi", bufs=1))
    omp = ctx.enter_context(tc.tile_pool(name="omp", bufs=1))
    wgt = ctx.enter_context(tc.tile_pool(name="wgt", bufs=1))
    walp = ctx.enter_context(tc.tile_pool(name="walp", bufs=1))
    wbp = ctx.enter_context(tc.tile_pool(name="wbp", bufs=3))
    gat = ctx.enter_context(tc.tile_pool(name="gat", bufs=6))
    qp = ctx.enter_context(tc.tile_pool(name="qp", bufs=4))
    fup = ctx.enter_context(tc.tile_pool(name="fup", bufs=2))
    pso = ctx.enter_context(tc.tile_pool(name="pso", bufs=2, space="PSUM"))
    psd = ctx.enter_context(tc.tile_pool(name="psd", bufs=1, space="PSUM"))
    drp = ctx.enter_context(tc.tile_pool(name="drp", bufs=2, space="DRAM"))

    # ---- persistent loads (critical first; spread across SP/Act queues) --
    com_t = cst.tile([128, 2 * 9 * 2 * 48], BF16, tag="com")
    nc.sync.dma_start(com_t[:], dt["com_w"])
    com_v = com_t[:].rearrange("p (l t i o) -> p l t i o", l=2, t=9, i=2, o=48)

    fsh = []
    for h in range(2):
        fs = cst.tile([128, FSZ], BF16, tag=f"fsh{h}")
        nc.sync.dma_start(fs[:], dt["finit"][128 * h:128 * (h + 1), :])
        fsh.append(fs)

    byx_t = cst.tile([64, 2 * 480], F32, tag="byx")
    nc.scalar.dma_start(byx_t[:], dt["byx"])
    hi0_t = cst.tile([64, 2], F32, tag="hi0")
    nc.scalar.dma_start(hi0_t[:], dt["hi0"])
    comb_t = cst.tile([48, 2], F32, tag="comb")
    nc.scalar.dma_start(comb_t[:], dt["com_b"])
    sel_t = cst.tile([32, 32 * 128], BF16, tag="sel")
    nc.scalar.dma_start(sel_t[:], dt["sel"])
    sel_v = sel_t[:].rearrange("p (r o) -> p r o", r=32)
    pperm_t = cst.tile([32, 4 * 32], F32, tag="pperm")
    nc.scalar.dma_start(pperm_t[:], dt["pperm"])
    pperm_v = pperm_t[:].rearrange("p (v o) -> p v o", v=4)
    mperm_t = cst.tile([16, 2 * 32], F32, tag="mperm")
    nc.scalar.dma_start(mperm_t[:], dt["mperm"])
    mperm_v = mperm_t[:].rearrange("p (v o) -> p v o", v=2)
    ident_t = cst.tile([128, 128], BF16, tag="ident")
    nc.scalar.dma_start(ident_t[:], dt["ident"])
    dcnb_t = cst.tile([128, 4], F32, tag="dcnb")
    nc.scalar.dma_start(dcnb_t[:], dt["dcn_b"])
    resb_t = cst.tile([128, 2], F32, tag="resb")
    nc.scalar.dma_start(resb_t[:], dt["res_b"])
    fh_t = cst.tile([128, 2 * RC], BF16, tag="fh")
    nc.scalar.dma_start(fh_t[:], dt["fh"])

    # per-level DCN weights, loaded once
    dcn_ts = []
    for lvl in range(2):
        t_ = cst.tile([128, NT * 2 * 2 * 128], BF16, tag=f"dcn{lvl}")
        nc.scalar.dma_start(t_[:], dt["dcn_w"][lvl])
        dcn_ts.append(t_[:].rearrange("p (k i o q) -> p k i o q",
                                      k=NT, i=2, o=2, q=128))

    fp_ap = {0: dt["fp1"], 1: dt["fp0"]}

    # DRAM staging tile for wrapped gather indices; zero it once so the
    # per-(t,z) pad lanes (cc 30:32) read as index 0 in every call.
    repD = drp.tile([16, NT * 64], I16, tag="repD")
    zs16 = smi.tile([16, NT * 64], I16, tag="zs16")
    nc.vector.memset(zs16[:], 0)
    nc.sync.dma_start(repD[:], zs16[:])

    # ---- per-call ---------------------------------------------------------
    for ci, lvl in enumerate(CALLS):
        Win = HIN[lvl]
        Wt = TW[lvl]
        dcn_v = dcn_ts[lvl]

        # offset conv: om_ps rows 0:48, (z,512)-chunked, 480 used
        om_ps = pso.tile([128, 1024], F32, tag="ps", name=f"omps_{ci}")
        conv3x3(nc, fsh, lambda ti, ih: com_v[:, lvl, ti, ih], om_ps, rows=48)

        # mask activation (com_b offset-bias is folded into byx host-side)
        m16 = omp.tile([16, RC], F32, tag="m16")
        omv1 = om_ps[32:48, :].rearrange("p (z c) -> p z c", z=2)[:, :, 0:480]
        nc.scalar.activation(m16[:], omv1, AF.Sigmoid,
                             bias=comb_t[32:48, lvl:lvl + 1])

        # stage offsets PSUM->SBUF, then shuffle into [64,480]
        # (p = yx*32 + rcb*16 + t) / [32,480] via PE permutation matmuls
        om01 = omp.tile([32, RC], F32, tag="om01")
        omv0 = om_ps[0:32, :].rearrange("p (z c) -> p z c", z=2)[:, :, 0:480]
        nc.scalar.activation(om01[:], omv0, AF.Copy)
        pos0ps = pso.tile([128, 1024], F32, tag="ps", name=f"pos0ps_{ci}")
        for yx in range(2):
            for rcb in range(2):
                nc.tensor.matmul(
                    pos0ps[yx * 32:(yx + 1) * 32, 0:480],
                    pperm_v[:, yx * 2 + rcb, :],
                    om01[0:32, rcb * 480:(rcb + 1) * 480],
                    start=(rcb == 0), stop=(rcb == 1))
        pos0 = pos0ps[0:64, 0:480]
        m32ps = pso.tile([128, 1024], F32, tag="ps", name=f"m32ps_{ci}")
        for rcb in range(2):
            nc.tensor.matmul(m32ps[0:32, 0:480], mperm_v[:, rcb, :],
                             m16[:, rcb * 480:(rcb + 1) * 480],
                             start=(rcb == 0), stop=(rcb == 1))
        m32 = m32ps[0:32, 0:480]

        # ---- small math ----
        cnt = [0]

        def t64():
            cnt[0] += 1
            return s64p.tile([64, 480], F32, tag="s64", name=f"t64_{ci}_{cnt[0]}")

        def t32():
            cnt[0] += 1
            return s32p.tile([32, 480], F32, tag="s32", name=f"t32_{ci}_{cnt[0]}")

        def t64i():
            cnt[0] += 1
            return s64p.tile([64, 480], I32, tag="s64i", bufs=1,
                             name=f"t64i_{ci}_{cnt[0]}")

        # positions carry a +1+1024 shift (baked into byx): +1 for the grid,
        # +1024 so floor-via-mod sees positive operands on hardware.
        # --- idx-critical path first (high priority: gathers wait on it) ---
        hp = tc.high_priority()
        hp.__enter__()
        sh = t64()
        nc.vector.tensor_tensor(sh[:], pos0,
                                byx_t[:, lvl * 480:(lvl + 1) * 480], A.add)
        i32t = t64i()
        nc.vector.tensor_copy(i32t[:], sh[:])
        ff = t64()
        nc.vector.tensor_copy(ff[:], i32t[:])
        gt = t64()
        nc.vector.tensor_tensor(gt[:], ff[:], sh[:], A.is_gt)
        fls = t64()
        nc.vector.tensor_tensor(fls[:], ff[:], gt[:], A.subtract)
        c0 = t64()
        nc.vector.tensor_scalar(c0[:], fls[:], 1024.0, hi0_t[:, lvl:lvl + 1],
                                A.max, A.min)
        # gather idx = (c0y-1024)*Wt + c0x-1024  (psx pre-subtracts the shift)
        psx = t32()
        nc.vector.tensor_scalar(psx[:], c0[32:64, :],
                                -1024.0 * (Wt + 1.0), None, A.add)
        gyt = t32()
        nc.vector.scalar_tensor_tensor(gyt[:], c0[0:32, :], float(Wt),
                                       psx[:], A.mult, A.add)
        i16t = smi.tile([32, 480], I16, tag="i16")
        nc.vector.tensor_copy(i16t[:], gyt[:])
        dflat = drp.tile([32, 480], I16, tag="dfl")
        nc.sync.dma_start(dflat[:], i16t[:])

        # idx wrap via DRAM: repD[p', t*64+z*32+cc] = dflat[(z*16+t)*480
        # + cc*16 + p'] (DRAM->DRAM strided, chunked by tap-half x z on two
        # queues), then one broadcast DMA fills all 8 replica row-groups.
        # repD pad lanes (cc 30:32) are zeroed once at kernel start.
        dfv = dflat[:].rearrange("p c -> (p c)")
        rdv = repD[:].rearrange("p (t z cc) -> p t z cc", t=NT, z=2, cc=32)
        HT = NT // 2
        for th, eng in ((0, nc.sync), (1, nc.scalar)):
            for z in range(2):
                wrap = smi.tile([16, HT * 30], I16, tag=f"wrap{th}{z}",
                                name=f"wrap_{ci}_{th}_{z}")
                base = (z * 16 + th * HT) * 480
                src = dfv[base:base + HT * 480]
                src = src.rearrange("(tc p) -> p tc", p=16)
                eng.dma_start(wrap[:], src)
                wv_ = wrap[:].rearrange("p (t cc) -> p t cc", t=HT)
                eng.dma_start(rdv[0:16, th * HT:(th + 1) * HT, z, 0:30], wv_)
        rep = smi.tile([128, NT * 64], I16, tag="rep")
        for grp in range(8):
            eng = nc.sync if grp % 2 == 0 else nc.scalar
            eng.dma_start(rep[grp * 16:(grp + 1) * 16, :], repD[:])
        hp.__exit__(None, None, None)

        # --- weight path (overlaps the idx DMA chain) ---
        frac = t64()
        nc.vector.tensor_tensor(frac[:], sh[:], fls[:], A.subtract)
        V = t64()
        nc.vector.tensor_tensor(V[:], c0[:], fls[:], A.is_equal)
        u = t64()
        nc.vector.tensor_scalar(u[:], frac[:], -1.0, 1.0, A.mult, A.add)

        # mask' = m * Vy * Vx  (x rows copied down to base partition 0;
        # weight-path copies on Act, off the DVE critical path)
        vx32 = t32()
        nc.scalar.copy(vx32[:], V[32:64, :])
        mv = t32()
        nc.vector.tensor_tensor(mv[:], m32, V[0:32, :], A.mult)
        mm_ = t32()
        nc.vector.tensor_tensor(mm_[:], mv[:], vx32[:], A.mult)
        A0 = t32()
        nc.vector.tensor_tensor(A0[:], u[0:32, :], mm_[:], A.mult)
        A1 = t32()
        nc.vector.tensor_tensor(A1[:], frac[0:32, :], mm_[:], A.mult)
        xs0 = t32()
        nc.scalar.copy(xs0[:], u[32:64, :])
        xs1 = t32()
        nc.scalar.copy(xs1[:], frac[32:64, :])

        # wall [32, (cy, px, 480)] bf16
        wall = walp.tile([32, 4 * 480], BF16, tag="wall")
        nc.vector.tensor_tensor(wall[:, 0 * 480:1 * 480], A0[:], xs0[:], A.mult)
        nc.vector.tensor_tensor(wall[:, 1 * 480:2 * 480], A0[:], xs1[:], A.mult)
        nc.vector.tensor_tensor(wall[:, 2 * 480:3 * 480], A1[:], xs0[:], A.mult)
        nc.vector.tensor_tensor(wall[:, 3 * 480:4 * 480], A1[:], xs1[:], A.mult)

        # dc accumulator [2][128, 1024] ((z,512)-chunked, 480 used)
        dcs = [psd.tile([128, 1024], F32, tag=f"dc{oh}", name=f"dc_{ci}_{oh}")
               for oh in range(2)]

        fpv = fp_ap[lvl]

        def emit_bcast(t):
            # PE broadcast via one-hot selector: bc[o,c] = wall[t+16z, c]
            wallb = wbp.tile([128, 4 * 960], BF16, tag="wallb",
                             name=f"wallb_{ci}_{t}")
            for j in range(4):
                bc = pso.tile([128, 1024], F32, tag="ps", name=f"bc_{ci}_{t}_{j}")
                for z in range(2):
                    nc.tensor.matmul(bc[:, z * 512:z * 512 + 480],
                                     sel_v[:, t + 16 * z, :],
                                     wall[0:32, j * 480:(j + 1) * 480],
                                     start=True, stop=True)
                bcv = bc[:].rearrange("p (z c) -> p z c", z=2)[:, :, 0:480]
                wbv = wallb[:, j * 960:(j + 1) * 960].rearrange(
                    "p (z c) -> p z c", z=2)
                nc.scalar.activation(wbv, bcv, AF.Copy)
            return wallb

        def emit_gather(t, z):
            # gather: one 2KB element per sample = full 2x2 patch; half-tap
            g = gat.tile([128, 8 * 512], BF16, tag="g", name=f"g_{ci}_{t}_{z}")
            gv = g[:].rearrange("p (j i) -> p j i", j=8)
            nc.gpsimd.dma_gather(gv, fpv,
                                 rep[:, t * 64 + z * 32:t * 64 + z * 32 + 32],
                                 512, 512, 1024, transpose=True,
                                 single_packet=False)
            return g

        # all gather dispatches up-front: Pool's in-order queue paces them
        # purely by gat-buffer WAR, never behind a compute op
        gs = {(t, z): emit_gather(t, z) for t in range(NT) for z in range(2)}
        wallbs = {0: emit_bcast(0)}
        for t in range(NT):
            if t + 1 < NT:
                wallbs[t + 1] = emit_bcast(t + 1)
            wallb = wallbs.pop(t)
            for z in range(2):
                g = gs.pop((t, z))
                gb = g[:]

                # in-place mul: p = g * wall  (one op, both corners)
                pv = vp(gb, [[4096, 128], [2048, 2], [1024, 2], [512, 2],
                             [1, 480]])
                wv = vp(wallb[:], [[3840, 128], [1920, 2], [960, 2], [0, 2],
                                   [1, 480]], doff=z * 480)
                nc.vector.tensor_tensor(pv, pv, wv, A.mult)

                # q = p[cy0] + p[cy1]   [128, (px, hl, 480)]
                # z0 on DVE, z1 on Pool
                q = qp.tile([128, 2 * 960], BF16, tag="q",
                            name=f"q_{ci}_{t}_{z}")
                qv = vp(q[:], [[1920, 128], [960, 2], [480, 2], [1, 480]])
                pa = vp(gb, [[4096, 128], [1024, 2], [512, 2], [1, 480]])
                pb = vp(gb, [[4096, 128], [1024, 2], [512, 2], [1, 480]],
                        doff=2048)
                nc.vector.tensor_tensor(qv, pa, pb, A.add)

                # s-sum folded into the matmuls: feed both px halves of q
                qview = q[:].rearrange("p (x h c) -> p x h c", x=2, h=2)
                for oh in range(2):
                    for ih in range(2):
                        for px in range(2):
                            nc.tensor.matmul(
                                dcs[oh][:, z * 512:z * 512 + 480],
                                dcn_v[:, t, ih, oh],
                                qview[:, px, ih, :],
                                start=(t == 0 and ih == 0 and px == 0),
                                stop=(t == NT - 1 and ih == 1 and px == 1))

        # f update: f += relu(dc + b)   (bf16 master; h1 add on Pool so the
        # two halves update in parallel and the conv starts sooner)
        for h in range(2):
            rel = fup.tile([128, RC], BF16, tag="rel", name=f"rel_{ci}_{h}")
            dcv = dcs[h][:].rearrange("p (z c) -> p z c", z=2)[:, :, 0:480]
            nc.scalar.activation(rel[:], dcv, AF.Relu,
                                 bias=dcnb_t[:, 2 * lvl + h:2 * lvl + h + 1])
            fsv = fsh[h][:].rearrange("p (r c) -> p r c", c=FW)[:, 1:25, 1:41]
            rv = rel[:].rearrange("p (r c) -> p r c", c=HOUT)
            (nc.vector if h == 0 else nc.gpsimd).tensor_tensor(
                fsv, fsv, rv, A.add)

    # ---- residual conv + fh ----------------------------------------------
    # fh is pre-accumulated into the PSUM via an identity matmul (start),
    # then the conv taps accumulate on top; output = act(psum + bias).
    res_t = wgt.tile([128, 9 * 2 * 2 * 128], BF16, tag="res")
    nc.sync.dma_start(res_t[:], dt["res_w"])
    res_v = res_t[:].rearrange("p (t i o q) -> p t i o q", t=9, i=2, o=2)
    for oh in range(2):
        rps = psd.tile([128, 1024], F32, tag=f"dc{oh}", name=f"rps_{oh}")
        fhv = fh_t[:].rearrange("p (o z c) -> p o z c", o=2, z=2)
        for z in range(2):
            nc.tensor.matmul(rps[:, z * 512:z * 512 + 480], ident_t[:],
                             fhv[:, oh, z, :], start=True, stop=False)
        conv3x3(nc, fsh, lambda ti, ih, oh=oh: res_v[:, ti, ih, oh], rps,
                accum=True)
        ot = fup.tile([128, RC], F32, tag="ot")
        rpv = rps[:].rearrange("p (z c) -> p z c", z=2)[:, :, 0:480]
        nc.scalar.activation(ot[:], rpv, AF.Identity, bias=resb_t[:, oh:oh + 1])
        nc.sync.dma_start(out_d[128 * oh:128 * (oh + 1), :], ot[:])


def conv3x3(nc, fsh, w_fn, out_ps, rows=128, accum=False):
    """3x3 stride-1 conv over the padded f window; out [rows, (z,512|480)].

    ih-outer so the ih=0 matmuls can start before fsh[1] is updated."""
    taps = [(a, b) for a in (-1, 0, 1) for b in (-1, 0, 1)]
    for ih in range(2):
        rhs = fsh[ih][:].rearrange("p (r c) -> p r c", c=FW)
        for ti, (dy, dx) in enumerate(taps):
            for nh in range(2):
                nc.tensor.matmul(
                    out_ps[0:rows, nh * 512:nh * 512 + 480],
                    w_fn(ti, ih),
                    rhs[:, 1 + dy + nh * 12:1 + dy + nh * 12 + 12,
                        1 + dx:1 + dx + 40],
                    start=(not accum and ih == 0 and ti == 0),
                    stop=(ih == 1 and ti == 8))


# ===========================================================================
# host side
# ===========================================================================

def packed_table(f):
    """[(H+1)*(W+1), 1024] bf16: entry (yy,xx) = 2x2 patch at (yy-1, xx-1)."""
    Cc, H, W = f.shape
    fpad = np.zeros((Cc, H + 2, W + 2), np.float32)
    fpad[:, 1:H + 1, 1:W + 1] = f
    parts = [fpad[:, dy:dy + H + 1, dx:dx + W + 1]
             for dy, dx in ((0, 0), (0, 1), (1, 0), (1, 1))]
    t = np.stack(parts, axis=0)            # [4, C, H+1, W+1]
    t = t.transpose(2, 3, 0, 1)            # [H+1, W+1, 4, C]
    return np.ascontiguousarray(
        t.reshape((H + 1) * (W + 1), 4 * Cc)).astype(ml_dtypes.bfloat16)


def prep_core_inputs(inputs, b, half):
    """Per-core input map for image b, row-half `half` (0=top)."""
    g0 = 0 if half == 0 else 16
    f0 = np.asarray(inputs["f0"][b], np.float32)
    f1 = np.asarray(inputs["f1"][b], np.float32)
    f2 = np.asarray(inputs["f2"][b], np.float32)

    finit = np.zeros((C, FR, FW), np.float32)
    for r in range(FR):
        gr = g0 - 1 + r
        if 0 <= gr < HOUT:
            finit[:, r, 1:41] = f2[:, gr, :]

    # fh as [128, (oh, rc)]
    fh0 = f2[:, g0:g0 + ROWS, :].reshape(C, RC)
    fh = np.concatenate([fh0[:128], fh0[128:]], axis=1)

    perm = list(range(0, 32, 2)) + list(range(1, 32, 2)) + list(range(32, 48))

    # base positions in +1-shifted grid coords; offset-conv bias folded in
    byx = np.zeros((2, 64, 480), np.float32)
    hi0 = np.zeros((2, 64, 1), np.float32)
    for lvl in range(2):
        k_, st_, pad_, dil_ = CONFIGS[lvl]
        Hin = HIN[lvl]
        cbp = np.asarray(inputs[f"com_b{lvl}"], np.float32)[perm]
        rc = np.arange(480)
        for rcb in range(2):
            rr = (rcb * 480 + rc) // HOUT
            cc = (rcb * 480 + rc) % HOUT
            for t in range(NT):
                byx[lvl, rcb * 16 + t] = (st_ * (g0 + rr) - pad_
                                          + (t // k_) * dil_ + 1025 + cbp[t])
                byx[lvl, 32 + rcb * 16 + t] = (st_ * cc - pad_ + (t % k_) * dil_
                                               + 1025 + cbp[16 + t])
        hi0[lvl, 0:32] = 1024 + Hin   # clamp hi in shifted coords
        hi0[lvl, 32:64] = 1024 + Hin
    byx = byx.transpose(1, 0, 2).reshape(64, 2 * 480)
    hi0 = hi0.transpose(1, 0, 2).reshape(64, 2)
    com_w = np.zeros((2, 9, 2, 128, 48), np.float32)
    com_b = np.zeros((2, 48, 1), np.float32)
    dcn_w = np.zeros((2, NT, 2, 2, 128, 128), np.float32)
    dcn_b = np.zeros((2, 2, 128, 1), np.float32)
    for lvl in range(2):
        cw = np.asarray(inputs[f"com_w{lvl}"], np.float32)[perm]
        cb = np.asarray(inputs[f"com_b{lvl}"], np.float32)[perm]
        for ty in range(3):
            for tx in range(3):
                for ih in range(2):
                    com_w[lvl, ty * 3 + tx, ih] = \
                        cw[:, ih * 128:(ih + 1) * 128, ty, tx].T
        com_b[lvl, :, 0] = cb
        dw = np.asarray(inputs[f"dcn_w{lvl}"], np.float32)
        for k in range(NT):
            for ih in range(2):
                for oh in range(2):
                    dcn_w[lvl, k, ih, oh] = dw[oh * 128:(oh + 1) * 128,
                                               ih * 128:(ih + 1) * 128,
                                               k // 4, k % 4].T
        db = np.asarray(inputs[f"dcn_b{lvl}"], np.float32)
        dcn_b[lvl, 0, :, 0] = db[:128]
        dcn_b[lvl, 1, :, 0] = db[128:]
    rw = np.asarray(inputs["res_w"], np.float32)
    res_w = np.zeros((9, 2, 2, 128, 128), np.float32)
    for ty in range(3):
        for tx in range(3):
            for ih in range(2):
                for oh in range(2):
                    res_w[ty * 3 + tx, ih, oh] = rw[oh * 128:(oh + 1) * 128,
                                                    ih * 128:(ih + 1) * 128,
                                                    ty, tx].T
    rb = np.asarray(inputs["res_b"], np.float32)
    res_b = np.stack([rb[:128], rb[128:]], axis=1)  # [128, 2]

    # PE permutation matrices for the om -> pos0 / m16 -> m32 shuffles
    pperm = np.zeros((32, 4, 32), np.float32)
    for yx in range(2):
        for rcb in range(2):
            for t in range(16):
                pperm[yx * 16 + t, yx * 2 + rcb, rcb * 16 + t] = 1.0
    mperm = np.zeros((16, 2, 32), np.float32)
    for rcb in range(2):
        for t in range(16):
            mperm[t, rcb, rcb * 16 + t] = 1.0

    com_w = com_w.transpose(3, 0, 1, 2, 4).reshape(128, -1)
    com_b = com_b.transpose(1, 0, 2).reshape(48, 2)
    dcn_w = dcn_w.transpose(0, 4, 1, 2, 3, 5).reshape(2, 128, -1)
    dcn_b = dcn_b.transpose(2, 0, 1, 3).reshape(128, 4)
    res_w = res_w.transpose(3, 0, 1, 2, 4).reshape(128, -1)

    return {
        "fp0": packed_table(f0),
        "fp1": packed_table(f1),
        "finit": finit.reshape(C, FSZ).astype(ml_dtypes.bfloat16),
        "fh": fh.astype(ml_dtypes.bfloat16),
        "byx": byx,
        "hi0": hi0,
        "sel": np.ascontiguousarray(
            np.tile(np.eye(32, dtype=np.float32)[:, :, None],
                    (1, 1, 128)).reshape(32, 32 * 128)
        ).astype(ml_dtypes.bfloat16),
        "pperm": pperm.reshape(32, 4 * 32),
        "mperm": mperm.reshape(16, 2 * 32),
        "ident": np.eye(128, dtype=np.float32).astype(ml_dtypes.bfloat16),
        "com_w": com_w.astype(ml_dtypes.bfloat16),
        "com_b": np.ascontiguousarray(com_b),
        "dcn_w": np.ascontiguousarray(dcn_w).astype(ml_dtypes.bfloat16),
        "dcn_b": np.ascontiguousarray(dcn_b),
        "res_w": np.ascontiguousarray(res_w).astype(ml_dtypes.bfloat16),
        "res_b": np.ascontiguousarray(res_b).astype(np.float32),
    }


def assemble_output(results):
    out = np.zeros((B, C, HOUT, HOUT), np.float32)
    for b in range(B):
        top = np.asarray(results[2 * b]["out"]).reshape(C, ROWS, HOUT)
        bot = np.asarray(results[2 * b + 1]["out"]).reshape(C, ROWS, HOUT)
        out[b, :, 0:20, :] = top[:, 0:20, :]
        out[b, :, 20:40, :] = bot[:, 4:24, :]
    return out


_NC_CACHE = []


def kernel(**inputs):
    if not _NC_CACHE:
        _NC_CACHE.append(build_program())
    nc = _NC_CACHE[0]
    in_maps = [prep_core_inputs(inputs, b, half)
               for b in range(B) for half in range(2)]
    from concourse.bass_utils import run_bass_kernel_spmd
    r = run_bass_kernel_spmd(nc, in_maps, list(range(8)))
    return assemble_output(r.results)
